# revision 36
# baseline (speedup 1.0000x reference)
"""Causal self-attention (B=2, S=2048, H=2048, 16 heads, hd=128) on 8 trn2 cores.

Sharding: tensor-parallel over heads (2 heads/core). Each core computes its
heads' QKV projection + RoPE + causal SDPA + a partial out-projection
(row-parallel w_out); the all-reduce over cores is done host-side as the
unshard step (sum of partials + b_out).

Device kernel (identical program on all cores, per-core weight data):
  phase 1: qkv = x @ w^T per head, q/k emitted head-major [d, t] via
           lhsT=w^T tiles, v emitted natural [t, d] via lhsT=x^T tiles.
           float32r matmuls (full PE rate, fp32 storage, 11-bit mantissa).
  rope:    even/odd dims pre-permuted into [evens;odds] rows host-side, so
           RoPE = pre*tab1 + swap(pre)*tab2 with a partition half-swap DMA.
  sdpa:    scores^T blocks [j,128 x i,512] = k^T.T @ q^T, exp on ScalarE,
           causal via N-restricted matmuls + triangular mask multiply,
           y^T accumulated in PSUM (lhsT=v_j), denominator via ones-matmul,
           normalize with reciprocal + K=1 broadcast matmul.
  phase 3: partial out [t,m] += y^T.T @ w_out^T slices, DMA to DRAM.

All matmul inputs are float32r: DRAM-sourced arrays are pre-rounded on the
host (RNE to 11-bit mantissa, low 12 bits zeroed); device-produced tiles are
declared f32r so ACT/DVE round on write; compute reads go through a .bitcast
back to f32.
"""

import sys

for _p in ("/opt/trn_rl_repo",):
    if _p not in sys.path:
        sys.path.append(_p)

import numpy as np

import concourse.bass as bass
import concourse.tile as tile
from concourse import bacc, mybir
from concourse.bass_utils import run_bass_kernel_spmd

B = 2
S = 2048
H = 2048
NH = 16
HD = 128
NCORES = 8
HPC = NH // NCORES          # heads per core = 2
T = B * S                   # 4096 flattened tokens
CHUNK = 256                 # phase-1 t-chunk
NCHUNK = S // CHUNK         # 8 per batch
F32 = mybir.dt.float32
F32R = mybir.dt.float32r
AF = mybir.ActivationFunctionType


def _f(ap):
    """View an f32r AP as plain f32 for compute-engine reads."""
    return ap.bitcast(F32)


def round_f32r(a: np.ndarray) -> np.ndarray:
    """Round fp32 to f32r (RNE to 11 explicit mantissa bits, low 12 bits 0)."""
    u = a.astype(np.float32).view(np.uint32)
    r = u + (0x7FF + ((u >> 12) & 1))
    return (r & np.uint32(0xFFFFF000)).view(np.float32)


def build_module():
    nc = bacc.Bacc("TRN2", target_bir_lowering=False, debug=False)

    xT_d = nc.dram_tensor("xT", [H, T], F32R, kind="ExternalInput").ap()
    wqk_d = nc.dram_tensor("wqkT", [H, 4 * HD], F32R, kind="ExternalInput").ap()
    wv_d = nc.dram_tensor("wvT", [H, 2 * HD], F32R, kind="ExternalInput").ap()
    wout_d = nc.dram_tensor("woutT", [2 * HD, H], F32R, kind="ExternalInput").ap()
    bqk_d = nc.dram_tensor("bqk", [128, 4], F32, kind="ExternalInput").ap()
    bv_d = nc.dram_tensor("bv", [128, 2 * HD], F32, kind="ExternalInput").ap()
    tab1_d = nc.dram_tensor("tab1", [128, S], F32, kind="ExternalInput").ap()
    tab2_d = nc.dram_tensor("tab2", [128, S], F32, kind="ExternalInput").ap()
    tri_d = nc.dram_tensor("tri", [128, 128], F32, kind="ExternalInput").ap()
    ones_d = nc.dram_tensor("ones", [128, 128], F32R, kind="ExternalInput").ap()
    out_d = nc.dram_tensor("outp", [T, H], mybir.dt.bfloat16, kind="ExternalOutput").ap()

    with tile.TileContext(nc) as tc:
        with (
            tc.tile_pool(name="consts", bufs=1) as consts,
            tc.tile_pool(name="qkpre", bufs=1) as qkpre_pool,
            tc.tile_pool(name="qkrot", bufs=1) as qkrot_pool,
            tc.tile_pool(name="vpool", bufs=1) as v_pool,
            tc.tile_pool(name="ytpool", bufs=1) as yt_pool,
            tc.tile_pool(name="xpool", bufs=4) as x_pool,
            tc.tile_pool(name="rope", bufs=2) as rope_pool,
            tc.tile_pool(name="ptpool", bufs=4) as pt_pool,
            tc.tile_pool(name="wostream", bufs=4) as wo_pool,
            tc.tile_pool(name="small", bufs=1) as small_pool,
            tc.tile_pool(name="accpool", bufs=1) as acc_pool,
            tc.tile_pool(name="ps256", bufs=2, space="PSUM") as ps256,
            tc.tile_pool(name="psA", bufs=4, space="PSUM") as psA,
            tc.tile_pool(name="psY", bufs=2, space="PSUM") as psY,
        ):
            # ---- resident constants ----
            wqk_s = consts.tile([128, 16 * 512], F32R, name="wqk_s")
            nc.sync.dma_start(
                wqk_s[:].rearrange("p (g o) -> p g o", g=16),
                wqk_d.rearrange("(g p) o -> p g o", p=128),
            )
            wv_s = consts.tile([128, 16 * 256], F32R, name="wv_s")
            nc.sync.dma_start(
                wv_s[:].rearrange("p (g o) -> p g o", g=16),
                wv_d.rearrange("(g p) o -> p g o", p=128),
            )
            bqk_s = consts.tile([128, 4], F32, name="bqk_s")
            nc.sync.dma_start(bqk_s[:], bqk_d[:])
            bv_s = consts.tile([128, 2 * HD], F32, name="bv_s")
            nc.sync.dma_start(bv_s[:], bv_d[:])
            # loaded later (first needed at rope of chunk 1 / SDPA) so the
            # startup DMA burst only covers wqk + wv + the first x chunk
            tab1_s = consts.tile([128, S], F32, name="tab1_s")
            tab2_s = consts.tile([128, S], F32, name="tab2_s")
            tri_s = consts.tile([128, 128], F32, name="tri_s")
            ones_s = consts.tile([128, 128], F32R, name="ones_s")

            def load_deferred_consts():
                nc.sync.dma_start(tab1_s[:], tab1_d[:])
                nc.sync.dma_start(tab2_s[:], tab2_d[:])
                nc.sync.dma_start(tri_s[:], tri_d[:])
                nc.sync.dma_start(ones_s[:], ones_d[:])

            for b in range(B):
                t0 = b * S

                # j order: q_h0, q_h1, k_h0, k_h1
                pre = [
                    qkpre_pool.tile([128, S], F32R, tag=f"pre{j}", name=f"pre{j}_{b}")
                    for j in range(4)
                ]
                rot = [
                    qkrot_pool.tile([128, S], F32R, tag=f"rot{j}", name=f"rot{j}_{b}")
                    for j in range(4)
                ]
                v_s = v_pool.tile([128, 16 * 256], F32R, tag="v", name=f"v_{b}")
                yt = [
                    yt_pool.tile([128, S], F32R, tag=f"yt{h}", name=f"yt{h}_{b}")
                    for h in range(HPC)
                ]

                # ---------------- phase 1: QKV projection ----------------
                for c in range(NCHUNK):
                    tc_off = t0 + CHUNK * c
                    xh = []
                    xT_3d = xT_d.rearrange("(g p) t -> p g t", p=128)
                    for qtr in range(4):
                        xt = x_pool.tile([128, 4 * CHUNK], F32R, tag="xc",
                                         name=f"xc_{b}_{c}_{qtr}")
                        nc.sync.dma_start(
                            xt[:].rearrange("p (g c) -> p g c", g=4),
                            xT_3d[:, 4 * qtr : 4 * (qtr + 1),
                                  tc_off : tc_off + CHUNK],
                        )
                        xh.append(xt)

                    for j in range(4):
                        ps = ps256.tile([128, CHUNK], F32, tag="p1",
                                        name=f"psqk_{b}_{c}_{j}")
                        for ht in range(16):
                            nc.tensor.matmul(
                                ps[:],
                                wqk_s[:, 512 * ht + 128 * j : 512 * ht + 128 * (j + 1)],
                                xh[ht // 4][:, CHUNK * (ht % 4) : CHUNK * (ht % 4 + 1)],
                                start=(ht == 0),
                                stop=(ht == 15),
                            )
                        nc.scalar.activation(
                            pre[j][:, CHUNK * c : CHUNK * (c + 1)],
                            ps[:],
                            AF.Identity,
                            bias=bqk_s[:, j : j + 1],
                        )

                    for tt in range(2):
                        gtt = 2 * c + tt  # t-tile within batch
                        psv = ps256.tile([128, 256], F32, tag="p1",
                                         name=f"psv_{b}_{c}_{tt}")
                        for ht in range(16):
                            nc.tensor.matmul(
                                psv[:],
                                xh[ht // 4][:, CHUNK * (ht % 4) + 128 * tt :
                                            CHUNK * (ht % 4) + 128 * (tt + 1)],
                                wv_s[:, 256 * ht : 256 * (ht + 1)],
                                start=(ht == 0),
                                stop=(ht == 15),
                            )
                        nc.vector.tensor_add(
                            v_s[:, 256 * gtt : 256 * (gtt + 1)], psv[:], bv_s[:]
                        )

                    if b == 0 and c == 0:
                        load_deferred_consts()

                    # rope per 512-column group, overlapped with phase 1
                    if c % 2 == 1:
                        cs = slice(CHUNK * (c - 1), CHUNK * (c + 1))
                        for j in range(4):
                            swap = rope_pool.tile([128, 512], F32R, tag="swap",
                                                  name=f"swap_{b}_{c}_{j}")
                            nc.scalar.dma_start(swap[0:64, :], pre[j][64:128, cs])
                            nc.scalar.dma_start(swap[64:128, :], pre[j][0:64, cs])
                            nc.vector.tensor_mul(
                                swap[:], _f(swap[:]), tab2_s[:, cs])
                            nc.vector.tensor_mul(
                                pre[j][:, cs], _f(pre[j][:, cs]), tab1_s[:, cs])
                            nc.vector.tensor_add(
                                rot[j][:, cs], _f(pre[j][:, cs]), _f(swap[:]))

                # ---------------- phase 2: SDPA per head ----------------
                for h in range(HPC):
                    qT = rot[h]
                    kT = rot[2 + h]
                    for ci in range(4):
                        i0 = 512 * ci
                        nj = 4 * ci + 4
                        ps_y = psY.tile([128, 512], F32, tag="y",
                                        name=f"psy_{b}_{h}_{ci}")
                        # denominator partials split across DVE (even jt,
                        # f32r) and GPSIMD (odd jt, f32 - POOL is idle)
                        acc = acc_pool.tile([128, 512], F32R, tag="acc",
                                            name=f"acc_{b}_{h}_{ci}")
                        accB = acc_pool.tile([128, 512], F32, tag="accB",
                                             name=f"accB_{b}_{h}_{ci}")
                        for jt in range(nj):
                            sub = jt - 4 * ci
                            s0 = max(0, 128 * sub)
                            pt = pt_pool.tile([128, 512], F32R, tag="pt",
                                              name=f"pt_{b}_{h}_{ci}_{jt}")
                            ps_s = psA.tile([128, 512], F32, tag="s",
                                            name=f"pss_{b}_{h}_{ci}_{jt}")
                            nc.tensor.matmul(
                                ps_s[:, s0:512],
                                kT[:, 128 * jt : 128 * (jt + 1)],
                                qT[:, i0 + s0 : i0 + 512],
                                start=True, stop=True,
                            )
                            nc.scalar.activation(
                                pt[:, s0:512], ps_s[:, s0:512], AF.Exp)
                            if sub >= 0:
                                nc.vector.tensor_mul(
                                    pt[:, s0 : s0 + 128],
                                    _f(pt[:, s0 : s0 + 128]),
                                    tri_s[:],
                                )
                            # jt==0 always has s0==0, so start=True initializes
                            # the full 512 columns; later partial-width matmuls
                            # accumulate into their valid subrange only.
                            nc.tensor.matmul(
                                ps_y[:, s0:512],
                                v_s[:, 256 * jt + 128 * h : 256 * jt + 128 * (h + 1)],
                                pt[:, s0:512],
                                start=(jt == 0), stop=(jt == nj - 1),
                            )
                            if jt == 0:
                                nc.vector.tensor_copy(acc[:], _f(pt[:]))
                            elif jt == 1:
                                if s0 > 0:
                                    nc.gpsimd.memset(accB[:, 0:s0], 0.0)
                                nc.gpsimd.tensor_copy(
                                    accB[:, s0:512], _f(pt[:, s0:512]))
                            elif jt % 2 == 0:
                                nc.vector.tensor_add(
                                    acc[:, s0:512],
                                    _f(acc[:, s0:512]),
                                    _f(pt[:, s0:512]),
                                )
                            else:
                                nc.gpsimd.tensor_add(
                                    accB[:, s0:512],
                                    accB[:, s0:512],
                                    _f(pt[:, s0:512]),
                                )
                        # normalize: yt = ps_y * (1/denom) broadcast
                        ps_dd = psA.tile([128, 512], F32, tag="s",
                                         name=f"psdd_{b}_{h}_{ci}")
                        nc.tensor.matmul(
                            ps_dd[0:1, :], ones_s[:, 0:1], acc[:],
                            start=True, stop=False,
                        )
                        nc.tensor.matmul(
                            ps_dd[0:1, :], _f(ones_s[:, 0:1]), accB[:],
                            start=False, stop=True,
                        )
                        rec = small_pool.tile([1, 512], F32R, tag="rec",
                                              name=f"rec_{b}_{h}_{ci}")
                        with nc.allow_low_precision("f32r matmul input"):
                            nc.vector.reciprocal(rec[:], ps_dd[0:1, :])
                        ps_b = psA.tile([128, 512], F32, tag="s",
                                        name=f"psb_{b}_{h}_{ci}")
                        nc.tensor.matmul(
                            ps_b[:], ones_s[0:1, :], rec[:],
                            start=True, stop=True,
                        )
                        ystage = pt_pool.tile([128, 512], F32, tag="pt",
                                              name=f"yst_{b}_{h}_{ci}")
                        nc.scalar.copy(ystage[:], ps_y[:])
                        nc.vector.tensor_mul(
                            yt[h][:, i0 : i0 + 512], ystage[:], ps_b[:]
                        )

                # ---------------- phase 3: out projection ----------------
                for mc in range(4):
                    m0 = 512 * mc
                    wo = [
                        wo_pool.tile([128, 512], F32R, tag="wo",
                                     name=f"wo_{b}_{mc}_{h}")
                        for h in range(HPC)
                    ]
                    for h in range(HPC):
                        nc.sync.dma_start(
                            wo[h][:], wout_d[128 * h : 128 * (h + 1), m0 : m0 + 512]
                        )
                    for tt in range(16):
                        ps_o = psA.tile([128, 512], F32, tag="s",
                                        name=f"pso_{b}_{mc}_{tt}")
                        for h in range(HPC):
                            nc.tensor.matmul(
                                ps_o[:],
                                yt[h][:, 128 * tt : 128 * (tt + 1)],
                                wo[h][:],
                                start=(h == 0), stop=(h == HPC - 1),
                            )
                        stage = pt_pool.tile([128, 512], mybir.dt.bfloat16,
                                             tag="st16", bufs=3,
                                             name=f"st_{b}_{mc}_{tt}")
                        if tt % 2 == 0:
                            nc.scalar.copy(stage[:], ps_o[:])
                        else:
                            nc.vector.tensor_copy(stage[:], ps_o[:])
                        nc.sync.dma_start(
                            out_d[t0 + 128 * tt : t0 + 128 * (tt + 1), m0 : m0 + 512],
                            stage[:],
                        )
    nc.compile()
    return nc


def _host_prep(x, w_qkv, b_qkv, w_out, b_out):
    """Build per-core input maps."""
    x2 = round_f32r(np.ascontiguousarray(x.reshape(T, H).T))  # [H, T]

    # rope tables (positions repeat per batch -> [128, S])
    inv = 10000.0 ** (-(np.arange(64, dtype=np.float64)) / 64.0)
    ang = np.arange(S, dtype=np.float64)[:, None] * inv[None, :]  # [S, 64]
    cos = np.cos(ang).T.astype(np.float32)  # [64, S]
    sin = np.sin(ang).T.astype(np.float32)
    tab1 = np.concatenate([cos, cos], axis=0)            # [128, S]
    tab2 = np.concatenate([-sin, sin], axis=0)           # [128, S]

    tri = np.triu(np.ones((128, 128), np.float32))       # [j, i] valid i>=j

    perm = np.concatenate([np.arange(0, 128, 2), np.arange(1, 128, 2)])
    scale = 1.0 / np.sqrt(HD)

    wq = w_qkv[0 * H : 1 * H].reshape(NH, HD, H)
    wk = w_qkv[1 * H : 2 * H].reshape(NH, HD, H)
    wv = w_qkv[2 * H : 3 * H].reshape(NH, HD, H)
    bq = b_qkv[0 * H : 1 * H].reshape(NH, HD)
    bk = b_qkv[1 * H : 2 * H].reshape(NH, HD)
    bv = b_qkv[2 * H : 3 * H].reshape(NH, HD)

    in_maps = []
    for c in range(NCORES):
        hs = [HPC * c + i for i in range(HPC)]
        cols = []
        bcols = []
        for h in hs:  # q heads (scaled + permuted)
            cols.append(wq[h][perm] * scale)
            bcols.append(bq[h][perm] * scale)
        for h in hs:  # k heads (permuted)
            cols.append(wk[h][perm])
            bcols.append(bk[h][perm])
        wqkT = round_f32r(
            np.ascontiguousarray(np.concatenate(cols, axis=0).T.astype(np.float32))
        )
        bqk = np.stack(bcols, axis=1).astype(np.float32)  # [128, 4]

        wvT = round_f32r(np.ascontiguousarray(
            np.concatenate([wv[h] for h in hs], axis=0).T.astype(np.float32)
        ))  # [H, 256]
        bvb = np.tile(
            np.concatenate([bv[h] for h in hs])[None, :], (128, 1)
        ).astype(np.float32)  # [128, 256]

        woutT = round_f32r(np.ascontiguousarray(
            w_out[:, HD * hs[0] : HD * (hs[-1] + 1)].T.astype(np.float32)
        ))  # [256, H]

        in_maps.append({
            "xT": x2,
            "wqkT": wqkT,
            "wvT": wvT,
            "woutT": woutT,
            "bqk": bqk,
            "bv": bvb,
            "tab1": tab1,
            "tab2": tab2,
            "tri": tri,
            "ones": np.ones((128, 128), np.float32),
        })
    return in_maps


_NC_CACHE = None


def get_module():
    global _NC_CACHE
    if _NC_CACHE is None:
        _NC_CACHE = build_module()
    return _NC_CACHE


def kernel(x, w_qkv, b_qkv, w_out, b_out):
    x = np.asarray(x, dtype=np.float32)
    w_qkv = np.asarray(w_qkv, dtype=np.float32)
    b_qkv = np.asarray(b_qkv, dtype=np.float32)
    w_out = np.asarray(w_out, dtype=np.float32)
    b_out = np.asarray(b_out, dtype=np.float32)

    nc = get_module()
    in_maps = _host_prep(x, w_qkv, b_qkv, w_out, b_out)
    res = run_bass_kernel_spmd(nc, in_maps, core_ids=list(range(NCORES)))
    acc = np.zeros((T, H), np.float64)
    for c in range(NCORES):
        acc += res.results[c]["outp"].astype(np.float64)
    out = (acc + b_out.astype(np.float64)[None, :]).astype(np.float32)
    return out.reshape(B, S, H)


# revision 37
# speedup vs baseline: 1.0314x; 1.0314x over previous
"""Causal self-attention (B=2, S=2048, H=2048, 16 heads, hd=128) on 8 trn2 cores.

Sharding: tensor-parallel over heads (2 heads/core). Each core computes its
heads' QKV projection + RoPE + causal SDPA + a partial out-projection
(row-parallel w_out); the all-reduce over cores is done host-side as the
unshard step (sum of partials + b_out).

Device kernel (identical program on all cores, per-core weight data):
  phase 1: qkv = x @ w^T per head, q/k emitted head-major [d, t] via
           lhsT=w^T tiles, v emitted natural [t, d] via lhsT=x^T tiles.
           float32r matmuls (full PE rate, fp32 storage, 11-bit mantissa).
  rope:    even/odd dims pre-permuted into [evens;odds] rows host-side, so
           RoPE = pre*tab1 + swap(pre)*tab2 with a partition half-swap DMA.
  sdpa:    scores^T blocks [j,128 x i,512] = k^T.T @ q^T, exp on ScalarE,
           causal via N-restricted matmuls + triangular mask multiply,
           y^T accumulated in PSUM (lhsT=v_j), denominator via ones-matmul,
           normalize with reciprocal + K=1 broadcast matmul.
  phase 3: partial out [t,m] += y^T.T @ w_out^T slices, DMA to DRAM.

All matmul inputs are float32r: DRAM-sourced arrays are pre-rounded on the
host (RNE to 11-bit mantissa, low 12 bits zeroed); device-produced tiles are
declared f32r so ACT/DVE round on write; compute reads go through a .bitcast
back to f32.
"""

import sys

for _p in ("/opt/trn_rl_repo",):
    if _p not in sys.path:
        sys.path.append(_p)

import numpy as np

import concourse.bass as bass
import concourse.tile as tile
from concourse import bacc, mybir
from concourse.bass_utils import run_bass_kernel_spmd

B = 2
S = 2048
H = 2048
NH = 16
HD = 128
NCORES = 8
HPC = NH // NCORES          # heads per core = 2
T = B * S                   # 4096 flattened tokens
CHUNK = 256                 # phase-1 t-chunk
NCHUNK = S // CHUNK         # 8 per batch
F32 = mybir.dt.float32
F32R = mybir.dt.float32r
AF = mybir.ActivationFunctionType


def _f(ap):
    """View an f32r AP as plain f32 for compute-engine reads."""
    return ap.bitcast(F32)


def round_f32r(a: np.ndarray) -> np.ndarray:
    """Round fp32 to f32r (RNE to 11 explicit mantissa bits, low 12 bits 0)."""
    u = a.astype(np.float32).view(np.uint32)
    r = u + (0x7FF + ((u >> 12) & 1))
    return (r & np.uint32(0xFFFFF000)).view(np.float32)


def build_module():
    nc = bacc.Bacc("TRN2", target_bir_lowering=False, debug=False)

    xT_d = nc.dram_tensor("xT", [H, T], F32R, kind="ExternalInput").ap()
    wqk_d = nc.dram_tensor("wqkT", [H, 4 * HD], F32R, kind="ExternalInput").ap()
    wv_d = nc.dram_tensor("wvT", [H, 2 * HD], F32R, kind="ExternalInput").ap()
    wout_d = nc.dram_tensor("woutT", [2 * HD, H], F32R, kind="ExternalInput").ap()
    bqk_d = nc.dram_tensor("bqk", [128, 4], F32, kind="ExternalInput").ap()
    bv_d = nc.dram_tensor("bv", [128, 2 * HD], F32, kind="ExternalInput").ap()
    tab1_d = nc.dram_tensor("tab1", [128, S], F32, kind="ExternalInput").ap()
    tab2_d = nc.dram_tensor("tab2", [128, S], F32, kind="ExternalInput").ap()
    tri_d = nc.dram_tensor("tri", [128, 128], F32, kind="ExternalInput").ap()
    ones_d = nc.dram_tensor("ones", [128, 128], F32R, kind="ExternalInput").ap()
    out_d = nc.dram_tensor("outp", [T, H], F32, kind="ExternalOutput").ap()

    with tile.TileContext(nc) as tc:
        with (
            tc.tile_pool(name="consts", bufs=1) as consts,
            tc.tile_pool(name="qkpre", bufs=1) as qkpre_pool,
            tc.tile_pool(name="qkrot", bufs=1) as qkrot_pool,
            tc.tile_pool(name="vpool", bufs=1) as v_pool,
            tc.tile_pool(name="ytpool", bufs=1) as yt_pool,
            tc.tile_pool(name="xpool", bufs=4) as x_pool,
            tc.tile_pool(name="rope", bufs=3) as rope_pool,
            tc.tile_pool(name="ptpool", bufs=4) as pt_pool,
            tc.tile_pool(name="wostream", bufs=4) as wo_pool,
            tc.tile_pool(name="small", bufs=1) as small_pool,
            tc.tile_pool(name="accpool", bufs=1) as acc_pool,
            tc.tile_pool(name="ps256", bufs=2, space="PSUM") as ps256,
            tc.tile_pool(name="psA", bufs=4, space="PSUM") as psA,
            tc.tile_pool(name="psY", bufs=2, space="PSUM") as psY,
        ):
            # ---- resident constants ----
            wqk_s = consts.tile([128, 16 * 512], F32R, name="wqk_s")
            nc.sync.dma_start(
                wqk_s[:].rearrange("p (g o) -> p g o", g=16),
                wqk_d.rearrange("(g p) o -> p g o", p=128),
            )
            wv_s = consts.tile([128, 16 * 256], F32R, name="wv_s")
            nc.sync.dma_start(
                wv_s[:].rearrange("p (g o) -> p g o", g=16),
                wv_d.rearrange("(g p) o -> p g o", p=128),
            )
            bqk_s = consts.tile([128, 4], F32, name="bqk_s")
            nc.sync.dma_start(bqk_s[:], bqk_d[:])
            bv_s = consts.tile([128, 2 * HD], F32, name="bv_s")
            nc.sync.dma_start(bv_s[:], bv_d[:])
            # loaded later (first needed at rope of chunk 1 / SDPA) so the
            # startup DMA burst only covers wqk + wv + the first x chunk
            tab1_s = consts.tile([128, S], F32, name="tab1_s")
            tab2_s = consts.tile([128, S], F32, name="tab2_s")
            tri_s = consts.tile([128, 128], F32, name="tri_s")
            ones_s = consts.tile([128, 128], F32R, name="ones_s")

            def load_deferred_consts():
                nc.sync.dma_start(tab1_s[:], tab1_d[:])
                nc.sync.dma_start(tab2_s[:], tab2_d[:])
                nc.sync.dma_start(tri_s[:], tri_d[:])
                nc.sync.dma_start(ones_s[:], ones_d[:])

            for b in range(B):
                t0 = b * S

                # j order: q_h0, q_h1, k_h0, k_h1
                pre = [
                    qkpre_pool.tile([128, S], F32R, tag=f"pre{j}", name=f"pre{j}_{b}")
                    for j in range(4)
                ]
                rot = [
                    qkrot_pool.tile([128, S], F32R, tag=f"rot{j}", name=f"rot{j}_{b}")
                    for j in range(4)
                ]
                v_s = v_pool.tile([128, 16 * 256], F32R, tag="v", name=f"v_{b}")
                yt = [
                    yt_pool.tile([128, S], F32R, tag=f"yt{h}", name=f"yt{h}_{b}")
                    for h in range(HPC)
                ]

                # ---------------- phase 1: QKV projection ----------------
                for c in range(NCHUNK):
                    tc_off = t0 + CHUNK * c
                    xh = []
                    xT_3d = xT_d.rearrange("(g p) t -> p g t", p=128)
                    for qtr in range(4):
                        xt = x_pool.tile([128, 4 * CHUNK], F32R, tag="xc",
                                         name=f"xc_{b}_{c}_{qtr}")
                        nc.sync.dma_start(
                            xt[:].rearrange("p (g c) -> p g c", g=4),
                            xT_3d[:, 4 * qtr : 4 * (qtr + 1),
                                  tc_off : tc_off + CHUNK],
                        )
                        xh.append(xt)

                    for j in range(4):
                        ps = ps256.tile([128, CHUNK], F32, tag="p1",
                                        name=f"psqk_{b}_{c}_{j}")
                        for ht in range(16):
                            nc.tensor.matmul(
                                ps[:],
                                wqk_s[:, 512 * ht + 128 * j : 512 * ht + 128 * (j + 1)],
                                xh[ht // 4][:, CHUNK * (ht % 4) : CHUNK * (ht % 4 + 1)],
                                start=(ht == 0),
                                stop=(ht == 15),
                            )
                        nc.scalar.activation(
                            pre[j][:, CHUNK * c : CHUNK * (c + 1)],
                            ps[:],
                            AF.Identity,
                            bias=bqk_s[:, j : j + 1],
                        )

                    for tt in range(2):
                        gtt = 2 * c + tt  # t-tile within batch
                        psv = ps256.tile([128, 256], F32, tag="p1",
                                         name=f"psv_{b}_{c}_{tt}")
                        for ht in range(16):
                            nc.tensor.matmul(
                                psv[:],
                                xh[ht // 4][:, CHUNK * (ht % 4) + 128 * tt :
                                            CHUNK * (ht % 4) + 128 * (tt + 1)],
                                wv_s[:, 256 * ht : 256 * (ht + 1)],
                                start=(ht == 0),
                                stop=(ht == 15),
                            )
                        nc.vector.tensor_add(
                            v_s[:, 256 * gtt : 256 * (gtt + 1)], psv[:], bv_s[:]
                        )

                    if b == 0 and c == 0:
                        load_deferred_consts()

                    # rope per 512-column group, overlapped with phase 1
                    if c % 2 == 1:
                        cs = slice(CHUNK * (c - 1), CHUNK * (c + 1))
                        for j in range(4):
                            swap = rope_pool.tile([128, 512], F32R, tag="swap",
                                                  name=f"swap_{b}_{c}_{j}")
                            nc.scalar.dma_start(swap[0:64, :], pre[j][64:128, cs])
                            nc.scalar.dma_start(swap[64:128, :], pre[j][0:64, cs])
                            nc.vector.tensor_mul(
                                swap[:], _f(swap[:]), tab2_s[:, cs])
                            nc.vector.tensor_mul(
                                pre[j][:, cs], _f(pre[j][:, cs]), tab1_s[:, cs])
                            nc.vector.tensor_add(
                                rot[j][:, cs], _f(pre[j][:, cs]), _f(swap[:]))

                # ---------------- phase 2: SDPA per head ----------------
                for h in range(HPC):
                    qT = rot[h]
                    kT = rot[2 + h]
                    for ci in range(4):
                        i0 = 512 * ci
                        nj = 4 * ci + 4
                        ps_y = psY.tile([128, 512], F32, tag="y",
                                        name=f"psy_{b}_{h}_{ci}")
                        # denominator partials split across DVE (even jt,
                        # f32r) and GPSIMD (odd jt, f32 - POOL is idle)
                        acc = acc_pool.tile([128, 512], F32R, tag="acc",
                                            name=f"acc_{b}_{h}_{ci}")
                        accB = acc_pool.tile([128, 512], F32, tag="accB",
                                             name=f"accB_{b}_{h}_{ci}")
                        for jt in range(nj):
                            sub = jt - 4 * ci
                            s0 = max(0, 128 * sub)
                            pt = pt_pool.tile([128, 512], F32R, tag="pt",
                                              name=f"pt_{b}_{h}_{ci}_{jt}")
                            ps_s = psA.tile([128, 512], F32, tag="s",
                                            name=f"pss_{b}_{h}_{ci}_{jt}")
                            nc.tensor.matmul(
                                ps_s[:, s0:512],
                                kT[:, 128 * jt : 128 * (jt + 1)],
                                qT[:, i0 + s0 : i0 + 512],
                                start=True, stop=True,
                            )
                            nc.scalar.activation(
                                pt[:, s0:512], ps_s[:, s0:512], AF.Exp)
                            if sub >= 0:
                                nc.vector.tensor_mul(
                                    pt[:, s0 : s0 + 128],
                                    _f(pt[:, s0 : s0 + 128]),
                                    tri_s[:],
                                )
                            # jt==0 always has s0==0, so start=True initializes
                            # the full 512 columns; later partial-width matmuls
                            # accumulate into their valid subrange only.
                            nc.tensor.matmul(
                                ps_y[:, s0:512],
                                v_s[:, 256 * jt + 128 * h : 256 * jt + 128 * (h + 1)],
                                pt[:, s0:512],
                                start=(jt == 0), stop=(jt == nj - 1),
                            )
                            if jt == 0:
                                nc.vector.tensor_copy(acc[:], _f(pt[:]))
                            elif jt == 1:
                                if s0 > 0:
                                    nc.gpsimd.memset(accB[:, 0:s0], 0.0)
                                nc.gpsimd.tensor_copy(
                                    accB[:, s0:512], _f(pt[:, s0:512]))
                            elif jt % 2 == 0:
                                nc.vector.tensor_add(
                                    acc[:, s0:512],
                                    _f(acc[:, s0:512]),
                                    _f(pt[:, s0:512]),
                                )
                            else:
                                nc.gpsimd.tensor_add(
                                    accB[:, s0:512],
                                    accB[:, s0:512],
                                    _f(pt[:, s0:512]),
                                )
                        # normalize: yt = ps_y * (1/denom) broadcast
                        ps_dd = psA.tile([128, 512], F32, tag="s",
                                         name=f"psdd_{b}_{h}_{ci}")
                        nc.tensor.matmul(
                            ps_dd[0:1, :], ones_s[:, 0:1], acc[:],
                            start=True, stop=False,
                        )
                        nc.tensor.matmul(
                            ps_dd[0:1, :], _f(ones_s[:, 0:1]), accB[:],
                            start=False, stop=True,
                        )
                        rec = small_pool.tile([1, 512], F32R, tag="rec",
                                              name=f"rec_{b}_{h}_{ci}")
                        with nc.allow_low_precision("f32r matmul input"):
                            nc.vector.reciprocal(rec[:], ps_dd[0:1, :])
                        ps_b = psA.tile([128, 512], F32, tag="s",
                                        name=f"psb_{b}_{h}_{ci}")
                        nc.tensor.matmul(
                            ps_b[:], ones_s[0:1, :], rec[:],
                            start=True, stop=True,
                        )
                        ystage = pt_pool.tile([128, 512], F32, tag="pt",
                                              name=f"yst_{b}_{h}_{ci}")
                        nc.scalar.copy(ystage[:], ps_y[:])
                        nc.vector.tensor_mul(
                            yt[h][:, i0 : i0 + 512], ystage[:], ps_b[:]
                        )

                # ---------------- phase 3: out projection ----------------
                for mc in range(4):
                    m0 = 512 * mc
                    wo = [
                        wo_pool.tile([128, 512], F32R, tag="wo",
                                     name=f"wo_{b}_{mc}_{h}")
                        for h in range(HPC)
                    ]
                    for h in range(HPC):
                        nc.sync.dma_start(
                            wo[h][:], wout_d[128 * h : 128 * (h + 1), m0 : m0 + 512]
                        )
                    for tt in range(16):
                        ps_o = psA.tile([128, 512], F32, tag="s",
                                        name=f"pso_{b}_{mc}_{tt}")
                        for h in range(HPC):
                            nc.tensor.matmul(
                                ps_o[:],
                                yt[h][:, 128 * tt : 128 * (tt + 1)],
                                wo[h][:],
                                start=(h == 0), stop=(h == HPC - 1),
                            )
                        stage = pt_pool.tile([128, 512], F32, tag="pt",
                                             name=f"st_{b}_{mc}_{tt}")
                        if tt % 2 == 0:
                            nc.scalar.copy(stage[:], ps_o[:])
                        else:
                            nc.vector.tensor_copy(stage[:], ps_o[:])
                        nc.sync.dma_start(
                            out_d[t0 + 128 * tt : t0 + 128 * (tt + 1), m0 : m0 + 512],
                            stage[:],
                        )
    nc.compile()
    return nc


def _host_prep(x, w_qkv, b_qkv, w_out, b_out):
    """Build per-core input maps."""
    x2 = round_f32r(np.ascontiguousarray(x.reshape(T, H).T))  # [H, T]

    # rope tables (positions repeat per batch -> [128, S])
    inv = 10000.0 ** (-(np.arange(64, dtype=np.float64)) / 64.0)
    ang = np.arange(S, dtype=np.float64)[:, None] * inv[None, :]  # [S, 64]
    cos = np.cos(ang).T.astype(np.float32)  # [64, S]
    sin = np.sin(ang).T.astype(np.float32)
    tab1 = np.concatenate([cos, cos], axis=0)            # [128, S]
    tab2 = np.concatenate([-sin, sin], axis=0)           # [128, S]

    tri = np.triu(np.ones((128, 128), np.float32))       # [j, i] valid i>=j

    perm = np.concatenate([np.arange(0, 128, 2), np.arange(1, 128, 2)])
    scale = 1.0 / np.sqrt(HD)

    wq = w_qkv[0 * H : 1 * H].reshape(NH, HD, H)
    wk = w_qkv[1 * H : 2 * H].reshape(NH, HD, H)
    wv = w_qkv[2 * H : 3 * H].reshape(NH, HD, H)
    bq = b_qkv[0 * H : 1 * H].reshape(NH, HD)
    bk = b_qkv[1 * H : 2 * H].reshape(NH, HD)
    bv = b_qkv[2 * H : 3 * H].reshape(NH, HD)

    in_maps = []
    for c in range(NCORES):
        hs = [HPC * c + i for i in range(HPC)]
        cols = []
        bcols = []
        for h in hs:  # q heads (scaled + permuted)
            cols.append(wq[h][perm] * scale)
            bcols.append(bq[h][perm] * scale)
        for h in hs:  # k heads (permuted)
            cols.append(wk[h][perm])
            bcols.append(bk[h][perm])
        wqkT = round_f32r(
            np.ascontiguousarray(np.concatenate(cols, axis=0).T.astype(np.float32))
        )
        bqk = np.stack(bcols, axis=1).astype(np.float32)  # [128, 4]

        wvT = round_f32r(np.ascontiguousarray(
            np.concatenate([wv[h] for h in hs], axis=0).T.astype(np.float32)
        ))  # [H, 256]
        bvb = np.tile(
            np.concatenate([bv[h] for h in hs])[None, :], (128, 1)
        ).astype(np.float32)  # [128, 256]

        woutT = round_f32r(np.ascontiguousarray(
            w_out[:, HD * hs[0] : HD * (hs[-1] + 1)].T.astype(np.float32)
        ))  # [256, H]

        in_maps.append({
            "xT": x2,
            "wqkT": wqkT,
            "wvT": wvT,
            "woutT": woutT,
            "bqk": bqk,
            "bv": bvb,
            "tab1": tab1,
            "tab2": tab2,
            "tri": tri,
            "ones": np.ones((128, 128), np.float32),
        })
    return in_maps


_NC_CACHE = None


def get_module():
    global _NC_CACHE
    if _NC_CACHE is None:
        _NC_CACHE = build_module()
    return _NC_CACHE


def kernel(x, w_qkv, b_qkv, w_out, b_out):
    x = np.asarray(x, dtype=np.float32)
    w_qkv = np.asarray(w_qkv, dtype=np.float32)
    b_qkv = np.asarray(b_qkv, dtype=np.float32)
    w_out = np.asarray(w_out, dtype=np.float32)
    b_out = np.asarray(b_out, dtype=np.float32)

    nc = get_module()
    in_maps = _host_prep(x, w_qkv, b_qkv, w_out, b_out)
    res = run_bass_kernel_spmd(nc, in_maps, core_ids=list(range(NCORES)))
    acc = np.zeros((T, H), np.float64)
    for c in range(NCORES):
        acc += res.results[c]["outp"].astype(np.float64)
    out = (acc + b_out.astype(np.float64)[None, :]).astype(np.float32)
    return out.reshape(B, S, H)


# revision 41
# speedup vs baseline: 1.0849x; 1.0519x over previous
"""Causal self-attention (B=2, S=2048, H=2048, 16 heads, hd=128) on 8 trn2 cores.

Sharding: tensor-parallel over heads (2 heads/core). Each core computes its
heads' QKV projection + RoPE + causal SDPA + a partial out-projection
(row-parallel w_out); the all-reduce over cores is done host-side as the
unshard step (sum of partials + b_out).

Device kernel (identical program on all cores, per-core weight data):
  phase 1: qkv = x @ w^T per head, q/k emitted head-major [d, t] via
           lhsT=w^T tiles, v emitted natural [t, d] via lhsT=x^T tiles.
           float32r matmuls (full PE rate, fp32 storage, 11-bit mantissa).
  rope:    even/odd dims pre-permuted into [evens;odds] rows host-side, so
           RoPE = pre*tab1 + swap(pre)*tab2 with a partition half-swap DMA.
  sdpa:    scores^T blocks [j,128 x i,512] = k^T.T @ q^T, exp on ScalarE,
           causal via N-restricted matmuls + triangular mask multiply,
           y^T accumulated in PSUM (lhsT=v_j), denominator via ones-matmul,
           normalize with reciprocal + K=1 broadcast matmul.
  phase 3: partial out [t,m] += y^T.T @ w_out^T slices, DMA to DRAM.

All matmul inputs are float32r: DRAM-sourced arrays are pre-rounded on the
host (RNE to 11-bit mantissa, low 12 bits zeroed); device-produced tiles are
declared f32r so ACT/DVE round on write; compute reads go through a .bitcast
back to f32.
"""

import sys

for _p in ("/opt/trn_rl_repo",):
    if _p not in sys.path:
        sys.path.append(_p)

import numpy as np

import concourse.bass as bass
import concourse.tile as tile
from concourse import bacc, mybir
from concourse.bass_utils import run_bass_kernel_spmd

B = 2
S = 2048
H = 2048
NH = 16
HD = 128
NCORES = 8
HPC = NH // NCORES          # heads per core = 2
T = B * S                   # 4096 flattened tokens
CHUNK = 256                 # phase-1 t-chunk
NCHUNK = S // CHUNK         # 8 per batch
F32 = mybir.dt.float32
F32R = mybir.dt.float32r
AF = mybir.ActivationFunctionType


def _f(ap):
    """View an f32r AP as plain f32 for compute-engine reads."""
    return ap.bitcast(F32)


def round_f32r(a: np.ndarray) -> np.ndarray:
    """Round fp32 to f32r (RNE to 11 explicit mantissa bits, low 12 bits 0)."""
    u = a.astype(np.float32).view(np.uint32)
    r = u + (0x7FF + ((u >> 12) & 1))
    return (r & np.uint32(0xFFFFF000)).view(np.float32)


def build_module():
    nc = bacc.Bacc("TRN2", target_bir_lowering=False, debug=False)

    xT_d = nc.dram_tensor("xT", [H, T], F32R, kind="ExternalInput").ap()
    wqk_d = nc.dram_tensor("wqkT", [H, 4 * HD], F32R, kind="ExternalInput").ap()
    wv_d = nc.dram_tensor("wvT", [H, 2 * HD], F32R, kind="ExternalInput").ap()
    wout_d = nc.dram_tensor("woutT", [2 * HD, H], F32R, kind="ExternalInput").ap()
    bqk_d = nc.dram_tensor("bqk", [128, 4], F32, kind="ExternalInput").ap()
    bv_d = nc.dram_tensor("bv", [128, 2 * HD], F32, kind="ExternalInput").ap()
    tab1_d = nc.dram_tensor("tab1", [128, S], F32, kind="ExternalInput").ap()
    tab2_d = nc.dram_tensor("tab2", [128, S], F32, kind="ExternalInput").ap()
    tri_d = nc.dram_tensor("tri", [128, 128], F32, kind="ExternalInput").ap()
    ones_d = nc.dram_tensor("ones", [128, 128], F32R, kind="ExternalInput").ap()
    out_d = nc.dram_tensor("outp", [T, H], F32, kind="ExternalOutput").ap()

    with tile.TileContext(nc) as tc:
        with (
            tc.tile_pool(name="consts", bufs=1) as consts,
            tc.tile_pool(name="qkpre", bufs=1) as qkpre_pool,
            tc.tile_pool(name="vpool", bufs=1) as v_pool,
            tc.tile_pool(name="ytpool", bufs=1) as yt_pool,
            tc.tile_pool(name="xpool", bufs=6) as x_pool,
            tc.tile_pool(name="rope", bufs=4) as rope_pool,
            tc.tile_pool(name="ptpool", bufs=6) as pt_pool,
            tc.tile_pool(name="small", bufs=1) as small_pool,
            tc.tile_pool(name="accpool", bufs=1) as acc_pool,
            tc.tile_pool(name="ps256", bufs=2, space="PSUM") as ps256,
            tc.tile_pool(name="psA", bufs=4, space="PSUM") as psA,
            tc.tile_pool(name="psY", bufs=2, space="PSUM") as psY,
        ):
            # ---- resident constants ----
            wqk_s = consts.tile([128, 16 * 512], F32R, name="wqk_s")
            for wh in range(2):
                nc.sync.dma_start(
                    wqk_s[:, 4096 * wh : 4096 * (wh + 1)].rearrange(
                        "p (g o) -> p g o", g=8),
                    wqk_d.rearrange("(g p) o -> p g o", p=128)[
                        :, 8 * wh : 8 * (wh + 1), :],
                )
            wv_s = consts.tile([128, 16 * 256], F32R, name="wv_s")
            nc.sync.dma_start(
                wv_s[:].rearrange("p (g o) -> p g o", g=16),
                wv_d.rearrange("(g p) o -> p g o", p=128),
            )
            bqk_s = consts.tile([128, 4], F32, name="bqk_s")
            nc.sync.dma_start(bqk_s[:], bqk_d[:])
            bv_s = consts.tile([128, 2 * HD], F32, name="bv_s")
            nc.sync.dma_start(bv_s[:], bv_d[:])
            # loaded later (first needed at rope of chunk 1 / SDPA) so the
            # startup DMA burst only covers wqk + wv + the first x chunk
            tab1_s = consts.tile([128, S], F32, name="tab1_s")
            tab2_s = consts.tile([128, S], F32, name="tab2_s")
            tri_s = consts.tile([128, 128], F32, name="tri_s")
            ones_s = consts.tile([128, 128], F32R, name="ones_s")
            wo_s = consts.tile([128, 2 * H], F32R, name="wo_s")

            def load_deferred_consts():
                nc.sync.dma_start(
                    wo_s[:].rearrange("p (g m) -> p g m", g=2),
                    wout_d.rearrange("(g p) m -> p g m", p=128),
                )
                nc.sync.dma_start(tab1_s[:], tab1_d[:])
                nc.sync.dma_start(tab2_s[:], tab2_d[:])
                nc.sync.dma_start(tri_s[:], tri_d[:])
                nc.sync.dma_start(ones_s[:], ones_d[:])

            for b in range(B):
                t0 = b * S

                # j order: q_h0, q_h1, k_h0, k_h1
                pre = [
                    qkpre_pool.tile([128, S], F32R, tag=f"pre{j}", name=f"pre{j}_{b}")
                    for j in range(4)
                ]
                v_s = v_pool.tile([128, 16 * 256], F32R, tag="v", name=f"v_{b}")
                yt = [
                    yt_pool.tile([128, S], F32R, tag=f"yt{h}", name=f"yt{h}_{b}")
                    for h in range(HPC)
                ]

                # ---------------- phase 1: QKV projection ----------------
                def emit_chunk(c, b=b, t0=t0, pre=pre, v_s=v_s):
                    tc_off = t0 + CHUNK * c
                    xh = []
                    xT_3d = xT_d.rearrange("(g p) t -> p g t", p=128)
                    for qtr in range(4):
                        xt = x_pool.tile([128, 4 * CHUNK], F32R, tag="xc",
                                         name=f"xc_{b}_{c}_{qtr}")
                        nc.sync.dma_start(
                            xt[:].rearrange("p (g c) -> p g c", g=4),
                            xT_3d[:, 4 * qtr : 4 * (qtr + 1),
                                  tc_off : tc_off + CHUNK],
                        )
                        xh.append(xt)

                    for j in range(4):
                        ps = ps256.tile([128, CHUNK], F32, tag="p1",
                                        name=f"psqk_{b}_{c}_{j}")
                        for ht in range(16):
                            nc.tensor.matmul(
                                ps[:],
                                wqk_s[:, 512 * ht + 128 * j : 512 * ht + 128 * (j + 1)],
                                xh[ht // 4][:, CHUNK * (ht % 4) : CHUNK * (ht % 4 + 1)],
                                start=(ht == 0),
                                stop=(ht == 15),
                            )
                        nc.scalar.activation(
                            pre[j][:, CHUNK * c : CHUNK * (c + 1)],
                            ps[:],
                            AF.Identity,
                            bias=bqk_s[:, j : j + 1],
                        )

                    for tt in range(2):
                        gtt = 2 * c + tt  # t-tile within batch
                        psv = ps256.tile([128, 256], F32, tag="p1",
                                         name=f"psv_{b}_{c}_{tt}")
                        for ht in range(16):
                            nc.tensor.matmul(
                                psv[:],
                                xh[ht // 4][:, CHUNK * (ht % 4) + 128 * tt :
                                            CHUNK * (ht % 4) + 128 * (tt + 1)],
                                wv_s[:, 256 * ht : 256 * (ht + 1)],
                                start=(ht == 0),
                                stop=(ht == 15),
                            )
                        nc.vector.tensor_add(
                            v_s[:, 256 * gtt : 256 * (gtt + 1)], psv[:], bv_s[:]
                        )

                    if b == 0 and c == 0:
                        load_deferred_consts()

                    # rope per 512-column group, overlapped with phase 1
                    if c % 2 == 1:
                        cs = slice(CHUNK * (c - 1), CHUNK * (c + 1))
                        for j in range(4):
                            swap = rope_pool.tile([128, 512], F32R, tag="swap",
                                                  name=f"swap_{b}_{c}_{j}")
                            nc.scalar.dma_start(swap[0:64, :], pre[j][64:128, cs])
                            nc.scalar.dma_start(swap[64:128, :], pre[j][0:64, cs])
                            nc.vector.tensor_mul(
                                swap[:], _f(swap[:]), tab2_s[:, cs])
                            nc.vector.tensor_mul(
                                pre[j][:, cs], _f(pre[j][:, cs]), tab1_s[:, cs])
                            nc.vector.tensor_add(
                                pre[j][:, cs], _f(pre[j][:, cs]), _f(swap[:]))

                # ---------------- phase 2: SDPA (interleaved per i-group) ---
                def emit_sdpa(h, ci, b=b, rot=pre, v_s=v_s, yt=yt):
                    qT = rot[h]
                    kT = rot[2 + h]
                    if True:
                        i0 = 512 * ci
                        nj = 4 * ci + 4
                        ps_y = psY.tile([128, 512], F32, tag="y",
                                        name=f"psy_{b}_{h}_{ci}")
                        # denominator partials split across DVE (even jt,
                        # f32r) and GPSIMD (odd jt, f32 - POOL is idle)
                        acc = acc_pool.tile([128, 512], F32R, tag="acc",
                                            name=f"acc_{b}_{h}_{ci}")
                        accB = acc_pool.tile([128, 512], F32, tag="accB",
                                             name=f"accB_{b}_{h}_{ci}")
                        for jt in range(nj):
                            sub = jt - 4 * ci
                            s0 = max(0, 128 * sub)
                            pt = pt_pool.tile([128, 512], F32R, tag="pt",
                                              name=f"pt_{b}_{h}_{ci}_{jt}")
                            ps_s = psA.tile([128, 512], F32, tag="s",
                                            name=f"pss_{b}_{h}_{ci}_{jt}")
                            nc.tensor.matmul(
                                ps_s[:, s0:512],
                                kT[:, 128 * jt : 128 * (jt + 1)],
                                qT[:, i0 + s0 : i0 + 512],
                                start=True, stop=True,
                            )
                            nc.scalar.activation(
                                pt[:, s0:512], ps_s[:, s0:512], AF.Exp)
                            if sub >= 0:
                                nc.vector.tensor_mul(
                                    pt[:, s0 : s0 + 128],
                                    _f(pt[:, s0 : s0 + 128]),
                                    tri_s[:],
                                )
                            # jt==0 always has s0==0, so start=True initializes
                            # the full 512 columns; later partial-width matmuls
                            # accumulate into their valid subrange only.
                            nc.tensor.matmul(
                                ps_y[:, s0:512],
                                v_s[:, 256 * jt + 128 * h : 256 * jt + 128 * (h + 1)],
                                pt[:, s0:512],
                                start=(jt == 0), stop=(jt == nj - 1),
                            )
                            if jt == 0:
                                nc.vector.tensor_copy(acc[:], _f(pt[:]))
                            elif jt == 1:
                                if s0 > 0:
                                    nc.gpsimd.memset(accB[:, 0:s0], 0.0)
                                nc.gpsimd.tensor_copy(
                                    accB[:, s0:512], _f(pt[:, s0:512]))
                            elif jt % 2 == 0:
                                nc.vector.tensor_add(
                                    acc[:, s0:512],
                                    _f(acc[:, s0:512]),
                                    _f(pt[:, s0:512]),
                                )
                            else:
                                nc.gpsimd.tensor_add(
                                    accB[:, s0:512],
                                    accB[:, s0:512],
                                    _f(pt[:, s0:512]),
                                )
                        # normalize: yt = ps_y * (1/denom) broadcast
                        ps_dd = psA.tile([128, 512], F32, tag="s",
                                         name=f"psdd_{b}_{h}_{ci}")
                        nc.tensor.matmul(
                            ps_dd[0:1, :], ones_s[:, 0:1], acc[:],
                            start=True, stop=False,
                        )
                        nc.tensor.matmul(
                            ps_dd[0:1, :], _f(ones_s[:, 0:1]), accB[:],
                            start=False, stop=True,
                        )
                        rec = small_pool.tile([1, 512], F32R, tag="rec",
                                              name=f"rec_{b}_{h}_{ci}")
                        with nc.allow_low_precision("f32r matmul input"):
                            nc.vector.reciprocal(rec[:], ps_dd[0:1, :])
                        ps_b = psA.tile([128, 512], F32, tag="s",
                                        name=f"psb_{b}_{h}_{ci}")
                        nc.tensor.matmul(
                            ps_b[:], ones_s[0:1, :], rec[:],
                            start=True, stop=True,
                        )
                        ystage = pt_pool.tile([128, 512], F32, tag="pt",
                                              name=f"yst_{b}_{h}_{ci}")
                        nc.scalar.copy(ystage[:], ps_y[:])
                        nc.vector.tensor_mul(
                            yt[h][:, i0 : i0 + 512], ystage[:], ps_b[:]
                        )

                # phase 3 (interleaved): out projection for a tt group
                def emit_outproj(ci, b=b, t0=t0, yt=yt):
                    for tt in range(4 * ci, 4 * ci + 4):
                        for mc in range(4):
                            m0 = 512 * mc
                            ps_o = psA.tile([128, 512], F32, tag="s",
                                            name=f"pso_{b}_{tt}_{mc}")
                            for h in range(HPC):
                                nc.tensor.matmul(
                                    ps_o[:],
                                    yt[h][:, 128 * tt : 128 * (tt + 1)],
                                    wo_s[:, 2048 * h + m0 : 2048 * h + m0 + 512],
                                    start=(h == 0), stop=(h == HPC - 1),
                                )
                            stage = pt_pool.tile([128, 512], F32, tag="pt",
                                                 name=f"st_{b}_{tt}_{mc}")
                            if mc % 2 == 0:
                                nc.scalar.copy(stage[:], ps_o[:])
                            else:
                                nc.vector.tensor_copy(stage[:], ps_o[:])
                            nc.sync.dma_start(
                                out_d[t0 + 128 * tt : t0 + 128 * (tt + 1),
                                      m0 : m0 + 512],
                                stage[:],
                            )

                # interleave: two phase-1 chunks + rope group, then both
                # heads' SDPA on the freshly completed i-group, then the
                # out-projection rows that group unlocked
                for ci in range(4):
                    emit_chunk(2 * ci)
                    emit_chunk(2 * ci + 1)
                    for h in range(HPC):
                        emit_sdpa(h, ci)
                    emit_outproj(ci)
    nc.compile()
    return nc


def _host_prep(x, w_qkv, b_qkv, w_out, b_out):
    """Build per-core input maps."""
    x2 = round_f32r(np.ascontiguousarray(x.reshape(T, H).T))  # [H, T]

    # rope tables (positions repeat per batch -> [128, S])
    inv = 10000.0 ** (-(np.arange(64, dtype=np.float64)) / 64.0)
    ang = np.arange(S, dtype=np.float64)[:, None] * inv[None, :]  # [S, 64]
    cos = np.cos(ang).T.astype(np.float32)  # [64, S]
    sin = np.sin(ang).T.astype(np.float32)
    tab1 = np.concatenate([cos, cos], axis=0)            # [128, S]
    tab2 = np.concatenate([-sin, sin], axis=0)           # [128, S]

    tri = np.triu(np.ones((128, 128), np.float32))       # [j, i] valid i>=j

    perm = np.concatenate([np.arange(0, 128, 2), np.arange(1, 128, 2)])
    scale = 1.0 / np.sqrt(HD)

    wq = w_qkv[0 * H : 1 * H].reshape(NH, HD, H)
    wk = w_qkv[1 * H : 2 * H].reshape(NH, HD, H)
    wv = w_qkv[2 * H : 3 * H].reshape(NH, HD, H)
    bq = b_qkv[0 * H : 1 * H].reshape(NH, HD)
    bk = b_qkv[1 * H : 2 * H].reshape(NH, HD)
    bv = b_qkv[2 * H : 3 * H].reshape(NH, HD)

    in_maps = []
    for c in range(NCORES):
        hs = [HPC * c + i for i in range(HPC)]
        cols = []
        bcols = []
        for h in hs:  # q heads (scaled + permuted)
            cols.append(wq[h][perm] * scale)
            bcols.append(bq[h][perm] * scale)
        for h in hs:  # k heads (permuted)
            cols.append(wk[h][perm])
            bcols.append(bk[h][perm])
        wqkT = round_f32r(
            np.ascontiguousarray(np.concatenate(cols, axis=0).T.astype(np.float32))
        )
        bqk = np.stack(bcols, axis=1).astype(np.float32)  # [128, 4]

        wvT = round_f32r(np.ascontiguousarray(
            np.concatenate([wv[h] for h in hs], axis=0).T.astype(np.float32)
        ))  # [H, 256]
        bvb = np.tile(
            np.concatenate([bv[h] for h in hs])[None, :], (128, 1)
        ).astype(np.float32)  # [128, 256]

        woutT = round_f32r(np.ascontiguousarray(
            w_out[:, HD * hs[0] : HD * (hs[-1] + 1)].T.astype(np.float32)
        ))  # [256, H]

        in_maps.append({
            "xT": x2,
            "wqkT": wqkT,
            "wvT": wvT,
            "woutT": woutT,
            "bqk": bqk,
            "bv": bvb,
            "tab1": tab1,
            "tab2": tab2,
            "tri": tri,
            "ones": np.ones((128, 128), np.float32),
        })
    return in_maps


_NC_CACHE = None


def get_module():
    global _NC_CACHE
    if _NC_CACHE is None:
        _NC_CACHE = build_module()
    return _NC_CACHE


def kernel(x, w_qkv, b_qkv, w_out, b_out):
    x = np.asarray(x, dtype=np.float32)
    w_qkv = np.asarray(w_qkv, dtype=np.float32)
    b_qkv = np.asarray(b_qkv, dtype=np.float32)
    w_out = np.asarray(w_out, dtype=np.float32)
    b_out = np.asarray(b_out, dtype=np.float32)

    nc = get_module()
    in_maps = _host_prep(x, w_qkv, b_qkv, w_out, b_out)
    res = run_bass_kernel_spmd(nc, in_maps, core_ids=list(range(NCORES)))
    acc = np.zeros((T, H), np.float64)
    for c in range(NCORES):
        acc += res.results[c]["outp"].astype(np.float64)
    out = (acc + b_out.astype(np.float64)[None, :]).astype(np.float32)
    return out.reshape(B, S, H)


# revision 50
# speedup vs baseline: 1.1858x; 1.0930x over previous
"""Causal self-attention (B=2, S=2048, H=2048, 16 heads, hd=128) on 8 trn2 cores.

Sharding: tensor-parallel over heads (2 heads/core). Each core computes its
heads' QKV projection + RoPE + causal SDPA + a partial out-projection
(row-parallel w_out); the all-reduce over cores is done host-side as the
unshard step (sum of partials + b_out).

Device kernel (identical program on all cores, per-core weight data):
  phase 1: qkv = x @ w^T per head, q/k emitted head-major [d, t] via
           lhsT=w^T tiles, v emitted natural [t, d] via lhsT=x^T tiles.
           float32r matmuls (full PE rate, fp32 storage, 11-bit mantissa).
  rope:    even/odd dims pre-permuted into [evens;odds] rows host-side, so
           RoPE = pre*tab1 + swap(pre)*tab2 with a partition half-swap DMA.
  sdpa:    scores^T blocks [j,128 x i,512] = k^T.T @ q^T, exp on ScalarE,
           causal via N-restricted matmuls + triangular mask multiply,
           y^T accumulated in PSUM (lhsT=v_j), denominator via ones-matmul,
           normalize with reciprocal + K=1 broadcast matmul.
  phase 3: partial out [t,m] += y^T.T @ w_out^T slices, DMA to DRAM.

All matmul inputs are float32r: DRAM-sourced arrays are pre-rounded on the
host (RNE to 11-bit mantissa, low 12 bits zeroed); device-produced tiles are
declared f32r so ACT/DVE round on write; compute reads go through a .bitcast
back to f32.
"""

import sys

for _p in ("/opt/trn_rl_repo",):
    if _p not in sys.path:
        sys.path.append(_p)

import numpy as np

import concourse.bass as bass
import concourse.tile as tile
from concourse import bacc, mybir
from concourse.bass_utils import run_bass_kernel_spmd

B = 2
S = 2048
H = 2048
NH = 16
HD = 128
NCORES = 8
HPC = NH // NCORES          # heads per core = 2
T = B * S                   # 4096 flattened tokens
CHUNK = 256                 # phase-1 t-chunk
NCHUNK = S // CHUNK         # 8 per batch
F32 = mybir.dt.float32
F32R = mybir.dt.float32r
AF = mybir.ActivationFunctionType


def _f(ap):
    """View an f32r AP as plain f32 for compute-engine reads."""
    return ap.bitcast(F32)


def round_f32r(a: np.ndarray) -> np.ndarray:
    """Round fp32 to f32r (RNE to 11 explicit mantissa bits, low 12 bits 0)."""
    u = a.astype(np.float32).view(np.uint32)
    r = u + (0x7FF + ((u >> 12) & 1))
    return (r & np.uint32(0xFFFFF000)).view(np.float32)


def build_module():
    nc = bacc.Bacc("TRN2", target_bir_lowering=False, debug=False)

    xT_d = nc.dram_tensor("xT", [H, T], F32R, kind="ExternalInput").ap()
    wqk_d = nc.dram_tensor("wqkT", [H, 4 * HD], F32R, kind="ExternalInput").ap()
    wv_d = nc.dram_tensor("wvT", [H, 2 * HD], F32R, kind="ExternalInput").ap()
    wout_d = nc.dram_tensor("woutT", [2 * HD, H], F32R, kind="ExternalInput").ap()
    bqk_d = nc.dram_tensor("bqk", [128, 4], F32, kind="ExternalInput").ap()
    bv_d = nc.dram_tensor("bv", [128, 2 * HD], F32, kind="ExternalInput").ap()
    tab1_d = nc.dram_tensor("tab1", [128, S], F32, kind="ExternalInput").ap()
    tab2_d = nc.dram_tensor("tab2", [128, S], F32, kind="ExternalInput").ap()
    tri_d = nc.dram_tensor("tri", [128, 128], F32, kind="ExternalInput").ap()
    ones_d = nc.dram_tensor("ones", [128, 128], F32R, kind="ExternalInput").ap()
    out_d = nc.dram_tensor("outp", [T, H], F32, kind="ExternalOutput").ap()

    with tile.TileContext(nc) as tc:
        with (
            tc.tile_pool(name="consts", bufs=1) as consts,
            tc.tile_pool(name="qkpre", bufs=1) as qkpre_pool,
            tc.tile_pool(name="vpool", bufs=1) as v_pool,
            tc.tile_pool(name="ytpool", bufs=1) as yt_pool,
            tc.tile_pool(name="xpool", bufs=6) as x_pool,
            tc.tile_pool(name="rope", bufs=4) as rope_pool,
            tc.tile_pool(name="ptpool", bufs=6) as pt_pool,
            tc.tile_pool(name="small", bufs=1) as small_pool,
            tc.tile_pool(name="accpool", bufs=1) as acc_pool,
            tc.tile_pool(name="ps256", bufs=3, space="PSUM") as ps256,
            tc.tile_pool(name="psA", bufs=3, space="PSUM") as psA,
            tc.tile_pool(name="psY", bufs=2, space="PSUM") as psY,
        ):
            # ---- resident constants ----
            wqk_s = consts.tile([128, 16 * 512], F32R, name="wqk_s")

            def load_wqk_j(j):
                nc.sync.dma_start(
                    wqk_s[:].rearrange("p (g o) -> p g o", g=16)[
                        :, :, 128 * j : 128 * (j + 1)],
                    wqk_d.rearrange("(g p) o -> p g o", p=128)[
                        :, :, 128 * j : 128 * (j + 1)],
                )
            wv_s = consts.tile([128, 16 * 256], F32R, name="wv_s")
            bqk_s = consts.tile([128, 4], F32, name="bqk_s")
            bv_s = consts.tile([128, 2 * HD], F32, name="bv_s")
            # loaded later (first needed once chunk 0's matmuls are running)
            # so the startup DMA burst only covers wqk + the first x chunks
            tab1_s = consts.tile([128, S], F32, name="tab1_s")
            tab2_s = consts.tile([128, S], F32, name="tab2_s")
            tri_s = consts.tile([128, 128], F32, name="tri_s")
            ones_s = consts.tile([128, 128], F32R, name="ones_s")
            wo_s = consts.tile([128, 2 * H], F32R, name="wo_s")

            def load_consts_a():
                nc.sync.dma_start(bqk_s[:], bqk_d[:])
                nc.sync.dma_start(
                    wv_s[:].rearrange("p (g o) -> p g o", g=16),
                    wv_d.rearrange("(g p) o -> p g o", p=128),
                )
                nc.sync.dma_start(bv_s[:], bv_d[:])

            def load_consts_b():
                nc.sync.dma_start(tab1_s[:], tab1_d[:])
                nc.sync.dma_start(tab2_s[:], tab2_d[:])

            def load_consts_c():
                nc.sync.dma_start(tri_s[:], tri_d[:])
                nc.sync.dma_start(ones_s[:], ones_d[:])
                nc.sync.dma_start(
                    wo_s[:].rearrange("p (g m) -> p g m", g=2),
                    wout_d.rearrange("(g p) m -> p g m", p=128),
                )

            for b in range(B):
                t0 = b * S

                # j order: q_h0, q_h1, k_h0, k_h1
                pre = [
                    qkpre_pool.tile([128, S], F32R, tag=f"pre{j}", name=f"pre{j}_{b}")
                    for j in range(4)
                ]
                v_s = v_pool.tile([128, 16 * 256], F32R, tag="v", name=f"v_{b}")
                yt = [
                    yt_pool.tile([128, S], F32R, tag=f"yt{h}", name=f"yt{h}_{b}")
                    for h in range(HPC)
                ]

                # ---------------- phase 1: QKV projection ----------------
                def emit_x(c, b=b, t0=t0):
                    tc_off = t0 + CHUNK * c
                    xh = []
                    xT_3d = xT_d.rearrange("(g p) t -> p g t", p=128)
                    for qtr in range(4):
                        xt = x_pool.tile([128, 4 * CHUNK], F32R, tag="xc",
                                         name=f"xc_{b}_{c}_{qtr}")
                        nc.sync.dma_start(
                            xt[:].rearrange("p (g c) -> p g c", g=4),
                            xT_3d[:, 4 * qtr : 4 * (qtr + 1),
                                  tc_off : tc_off + CHUNK],
                        )
                        xh.append(xt)
                    return xh

                def emit_chunk(c, xh, b=b, t0=t0, pre=pre, v_s=v_s):
                    for j in range(4):
                        ps = ps256.tile([128, CHUNK], F32, tag="p1",
                                        name=f"psqk_{b}_{c}_{j}")
                        for ht in range(16):
                            nc.tensor.matmul(
                                ps[:],
                                wqk_s[:, 512 * ht + 128 * j : 512 * ht + 128 * (j + 1)],
                                xh[ht // 4][:, CHUNK * (ht % 4) : CHUNK * (ht % 4 + 1)],
                                start=(ht == 0),
                                stop=(ht == 15),
                            )
                        nc.scalar.activation(
                            pre[j][:, CHUNK * c : CHUNK * (c + 1)],
                            ps[:],
                            AF.Identity,
                            bias=bqk_s[:, j : j + 1],
                        )

                    for tt in range(2):
                        gtt = 2 * c + tt  # t-tile within batch
                        psv = ps256.tile([128, 256], F32, tag="p1",
                                         name=f"psv_{b}_{c}_{tt}")
                        for ht in range(16):
                            nc.tensor.matmul(
                                psv[:],
                                xh[ht // 4][:, CHUNK * (ht % 4) + 128 * tt :
                                            CHUNK * (ht % 4) + 128 * (tt + 1)],
                                wv_s[:, 256 * ht : 256 * (ht + 1)],
                                start=(ht == 0),
                                stop=(ht == 15),
                            )
                        nc.vector.tensor_add(
                            v_s[:, 256 * gtt : 256 * (gtt + 1)], psv[:], bv_s[:]
                        )

                    if b == 0 and c == 0:
                        load_consts_a()
                    if b == 0 and c == 1:
                        load_consts_b()

                    # rope per 512-column group, overlapped with phase 1
                    if c % 2 == 1:
                        cs = slice(CHUNK * (c - 1), CHUNK * (c + 1))
                        for j in range(4):
                            swap = rope_pool.tile([128, 512], F32R, tag="swap",
                                                  name=f"swap_{b}_{c}_{j}")
                            nc.scalar.dma_start(swap[0:64, :], pre[j][64:128, cs])
                            nc.scalar.dma_start(swap[64:128, :], pre[j][0:64, cs])
                            nc.vector.tensor_mul(
                                swap[:], _f(swap[:]), tab2_s[:, cs])
                            nc.vector.tensor_mul(
                                pre[j][:, cs], _f(pre[j][:, cs]), tab1_s[:, cs])
                            nc.vector.tensor_add(
                                pre[j][:, cs], _f(pre[j][:, cs]), _f(swap[:]))

                # ---------------- phase 2: SDPA (interleaved per i-group) ---
                def emit_sdpa(h, ci, b=b, rot=pre, v_s=v_s, yt=yt):
                    qT = rot[h]
                    kT = rot[2 + h]
                    if True:
                        i0 = 512 * ci
                        nj = 4 * ci + 4
                        ps_y = psY.tile([128, 512], F32, tag="y",
                                        name=f"psy_{b}_{h}_{ci}")
                        # denominator partials split across DVE (even jt,
                        # f32r) and GPSIMD (odd jt, f32 - POOL is idle)
                        acc = acc_pool.tile([128, 512], F32R, tag="acc",
                                            name=f"acc_{b}_{h}_{ci}")
                        accB = acc_pool.tile([128, 512], F32, tag="accB",
                                             name=f"accB_{b}_{h}_{ci}")
                        for jt in range(nj):
                            sub = jt - 4 * ci
                            s0 = max(0, 128 * sub)
                            pt = pt_pool.tile([128, 512], F32R, tag="pt",
                                              name=f"pt_{b}_{h}_{ci}_{jt}")
                            ps_s = psA.tile([128, 512], F32, tag="s",
                                            name=f"pss_{b}_{h}_{ci}_{jt}")
                            nc.tensor.matmul(
                                ps_s[:, s0:512],
                                kT[:, 128 * jt : 128 * (jt + 1)],
                                qT[:, i0 + s0 : i0 + 512],
                                start=True, stop=True,
                            )
                            nc.scalar.activation(
                                pt[:, s0:512], ps_s[:, s0:512], AF.Exp)
                            if sub >= 0:
                                nc.vector.tensor_mul(
                                    pt[:, s0 : s0 + 128],
                                    _f(pt[:, s0 : s0 + 128]),
                                    tri_s[:],
                                )
                            # jt==0 always has s0==0, so start=True initializes
                            # the full 512 columns; later partial-width matmuls
                            # accumulate into their valid subrange only.
                            nc.tensor.matmul(
                                ps_y[:, s0:512],
                                v_s[:, 256 * jt + 128 * h : 256 * jt + 128 * (h + 1)],
                                pt[:, s0:512],
                                start=(jt == 0), stop=(jt == nj - 1),
                            )
                            if jt == 0:
                                nc.vector.tensor_copy(acc[:], _f(pt[:]))
                            elif jt == 1:
                                if s0 > 0:
                                    nc.gpsimd.memset(accB[:, 0:s0], 0.0)
                                nc.gpsimd.tensor_copy(
                                    accB[:, s0:512], _f(pt[:, s0:512]))
                            elif jt % 2 == 0:
                                nc.vector.tensor_add(
                                    acc[:, s0:512],
                                    _f(acc[:, s0:512]),
                                    _f(pt[:, s0:512]),
                                )
                            else:
                                nc.gpsimd.tensor_add(
                                    accB[:, s0:512],
                                    accB[:, s0:512],
                                    _f(pt[:, s0:512]),
                                )
                        # normalize: yt = ps_y * (1/denom) broadcast
                        ps_dd = psA.tile([128, 512], F32, tag="s",
                                         name=f"psdd_{b}_{h}_{ci}")
                        nc.tensor.matmul(
                            ps_dd[0:1, :], ones_s[:, 0:1], acc[:],
                            start=True, stop=False,
                        )
                        nc.tensor.matmul(
                            ps_dd[0:1, :], _f(ones_s[:, 0:1]), accB[:],
                            start=False, stop=True,
                        )
                        rec = small_pool.tile([1, 512], F32R, tag="rec",
                                              name=f"rec_{b}_{h}_{ci}")
                        with nc.allow_low_precision("f32r matmul input"):
                            nc.vector.reciprocal(rec[:], ps_dd[0:1, :])
                        ps_b = psA.tile([128, 512], F32, tag="s",
                                        name=f"psb_{b}_{h}_{ci}")
                        nc.tensor.matmul(
                            ps_b[:], ones_s[0:1, :], rec[:],
                            start=True, stop=True,
                        )
                        ystage = pt_pool.tile([128, 512], F32, tag="pt",
                                              name=f"yst_{b}_{h}_{ci}")
                        nc.scalar.copy(ystage[:], ps_y[:])
                        nc.vector.tensor_mul(
                            yt[h][:, i0 : i0 + 512], ystage[:], ps_b[:]
                        )

                # phase 3 (interleaved): out projection for a tt group
                def emit_outproj(ci, b=b, t0=t0, yt=yt):
                    for tt in range(4 * ci, 4 * ci + 4):
                        for mc in range(4):
                            m0 = 512 * mc
                            ps_o = psA.tile([128, 512], F32, tag="s",
                                            name=f"pso_{b}_{tt}_{mc}")
                            for h in range(HPC):
                                nc.tensor.matmul(
                                    ps_o[:],
                                    yt[h][:, 128 * tt : 128 * (tt + 1)],
                                    wo_s[:, 2048 * h + m0 : 2048 * h + m0 + 512],
                                    start=(h == 0), stop=(h == HPC - 1),
                                )
                            stage = pt_pool.tile([128, 512], F32, tag="pt",
                                                 name=f"st_{b}_{tt}_{mc}")
                            if mc % 2 == 0:
                                nc.scalar.copy(stage[:], ps_o[:])
                            else:
                                nc.vector.tensor_copy(stage[:], ps_o[:])
                            eng = nc.gpsimd if mc % 2 == 0 else nc.scalar
                            eng.dma_start(
                                out_d[t0 + 128 * tt : t0 + 128 * (tt + 1),
                                      m0 : m0 + 512],
                                stage[:],
                            )

                # interleave: two phase-1 chunks + rope group, then both
                # heads' SDPA on the freshly completed i-group, then the
                # out-projection rows that group unlocked
                if b == 0:
                    for j in range(4):
                        load_wqk_j(j)
                xq = {0: emit_x(0), 1: emit_x(1)}
                for ci in range(4):
                    emit_chunk(2 * ci, xq.pop(2 * ci))
                    emit_chunk(2 * ci + 1, xq.pop(2 * ci + 1))
                    if ci < 3:
                        xq[2 * ci + 2] = emit_x(2 * ci + 2)
                        xq[2 * ci + 3] = emit_x(2 * ci + 3)
                    if b == 0 and ci == 0:
                        load_consts_c()
                    for h in range(HPC):
                        emit_sdpa(h, ci)
                    emit_outproj(ci)
    nc.compile()
    return nc


def _host_prep(x, w_qkv, b_qkv, w_out, b_out):
    """Build per-core input maps."""
    x2 = round_f32r(np.ascontiguousarray(x.reshape(T, H).T))  # [H, T]

    # rope tables (positions repeat per batch -> [128, S])
    inv = 10000.0 ** (-(np.arange(64, dtype=np.float64)) / 64.0)
    ang = np.arange(S, dtype=np.float64)[:, None] * inv[None, :]  # [S, 64]
    cos = np.cos(ang).T.astype(np.float32)  # [64, S]
    sin = np.sin(ang).T.astype(np.float32)
    tab1 = np.concatenate([cos, cos], axis=0)            # [128, S]
    tab2 = np.concatenate([-sin, sin], axis=0)           # [128, S]

    tri = np.triu(np.ones((128, 128), np.float32))       # [j, i] valid i>=j

    perm = np.concatenate([np.arange(0, 128, 2), np.arange(1, 128, 2)])
    scale = 1.0 / np.sqrt(HD)

    wq = w_qkv[0 * H : 1 * H].reshape(NH, HD, H)
    wk = w_qkv[1 * H : 2 * H].reshape(NH, HD, H)
    wv = w_qkv[2 * H : 3 * H].reshape(NH, HD, H)
    bq = b_qkv[0 * H : 1 * H].reshape(NH, HD)
    bk = b_qkv[1 * H : 2 * H].reshape(NH, HD)
    bv = b_qkv[2 * H : 3 * H].reshape(NH, HD)

    in_maps = []
    for c in range(NCORES):
        hs = [HPC * c + i for i in range(HPC)]
        cols = []
        bcols = []
        for h in hs:  # q heads (scaled + permuted)
            cols.append(wq[h][perm] * scale)
            bcols.append(bq[h][perm] * scale)
        for h in hs:  # k heads (permuted)
            cols.append(wk[h][perm])
            bcols.append(bk[h][perm])
        wqkT = round_f32r(
            np.ascontiguousarray(np.concatenate(cols, axis=0).T.astype(np.float32))
        )
        bqk = np.stack(bcols, axis=1).astype(np.float32)  # [128, 4]

        wvT = round_f32r(np.ascontiguousarray(
            np.concatenate([wv[h] for h in hs], axis=0).T.astype(np.float32)
        ))  # [H, 256]
        bvb = np.tile(
            np.concatenate([bv[h] for h in hs])[None, :], (128, 1)
        ).astype(np.float32)  # [128, 256]

        woutT = round_f32r(np.ascontiguousarray(
            w_out[:, HD * hs[0] : HD * (hs[-1] + 1)].T.astype(np.float32)
        ))  # [256, H]

        in_maps.append({
            "xT": x2,
            "wqkT": wqkT,
            "wvT": wvT,
            "woutT": woutT,
            "bqk": bqk,
            "bv": bvb,
            "tab1": tab1,
            "tab2": tab2,
            "tri": tri,
            "ones": np.ones((128, 128), np.float32),
        })
    return in_maps


_NC_CACHE = None


def get_module():
    global _NC_CACHE
    if _NC_CACHE is None:
        _NC_CACHE = build_module()
    return _NC_CACHE


def kernel(x, w_qkv, b_qkv, w_out, b_out):
    x = np.asarray(x, dtype=np.float32)
    w_qkv = np.asarray(w_qkv, dtype=np.float32)
    b_qkv = np.asarray(b_qkv, dtype=np.float32)
    w_out = np.asarray(w_out, dtype=np.float32)
    b_out = np.asarray(b_out, dtype=np.float32)

    nc = get_module()
    in_maps = _host_prep(x, w_qkv, b_qkv, w_out, b_out)
    res = run_bass_kernel_spmd(nc, in_maps, core_ids=list(range(NCORES)))
    acc = np.zeros((T, H), np.float64)
    for c in range(NCORES):
        acc += res.results[c]["outp"].astype(np.float64)
    out = (acc + b_out.astype(np.float64)[None, :]).astype(np.float32)
    return out.reshape(B, S, H)


# revision 52
# speedup vs baseline: 1.2090x; 1.0196x over previous
"""Causal self-attention (B=2, S=2048, H=2048, 16 heads, hd=128) on 8 trn2 cores.

Sharding: tensor-parallel over heads (2 heads/core). Each core computes its
heads' QKV projection + RoPE + causal SDPA + a partial out-projection
(row-parallel w_out); the all-reduce over cores is done host-side as the
unshard step (sum of partials + b_out).

Device kernel (identical program on all cores, per-core weight data):
  phase 1: qkv = x @ w^T per head, q/k emitted head-major [d, t] via
           lhsT=w^T tiles, v emitted natural [t, d] via lhsT=x^T tiles.
           float32r matmuls (full PE rate, fp32 storage, 11-bit mantissa).
  rope:    even/odd dims pre-permuted into [evens;odds] rows host-side, so
           RoPE = pre*tab1 + swap(pre)*tab2 with a partition half-swap DMA.
  sdpa:    scores^T blocks [j,128 x i,512] = k^T.T @ q^T, exp on ScalarE,
           causal via N-restricted matmuls + triangular mask multiply,
           y^T accumulated in PSUM (lhsT=v_j), denominator via ones-matmul,
           normalize with reciprocal + K=1 broadcast matmul.
  phase 3: partial out [t,m] += y^T.T @ w_out^T slices, DMA to DRAM.

All matmul inputs are float32r: DRAM-sourced arrays are pre-rounded on the
host (RNE to 11-bit mantissa, low 12 bits zeroed); device-produced tiles are
declared f32r so ACT/DVE round on write; compute reads go through a .bitcast
back to f32.
"""

import sys

for _p in ("/opt/trn_rl_repo",):
    if _p not in sys.path:
        sys.path.append(_p)

import numpy as np

import concourse.bass as bass
import concourse.tile as tile
from concourse import bacc, mybir
from concourse.bass_utils import run_bass_kernel_spmd

B = 2
S = 2048
H = 2048
NH = 16
HD = 128
NCORES = 8
HPC = NH // NCORES          # heads per core = 2
T = B * S                   # 4096 flattened tokens
CHUNK = 256                 # phase-1 t-chunk
NCHUNK = S // CHUNK         # 8 per batch
F32 = mybir.dt.float32
F32R = mybir.dt.float32r
AF = mybir.ActivationFunctionType


def _f(ap):
    """View an f32r AP as plain f32 for compute-engine reads."""
    return ap.bitcast(F32)


def round_f32r(a: np.ndarray) -> np.ndarray:
    """Round fp32 to f32r (RNE to 11 explicit mantissa bits, low 12 bits 0)."""
    u = a.astype(np.float32).view(np.uint32)
    r = u + (0x7FF + ((u >> 12) & 1))
    return (r & np.uint32(0xFFFFF000)).view(np.float32)


def build_module():
    nc = bacc.Bacc("TRN2", target_bir_lowering=False, debug=False)

    xT_d = nc.dram_tensor("xT", [H, T], F32R, kind="ExternalInput").ap()
    wqk_d = nc.dram_tensor("wqkT", [H, 4 * HD], F32R, kind="ExternalInput").ap()
    wv_d = nc.dram_tensor("wvT", [H, 2 * HD], F32R, kind="ExternalInput").ap()
    wout_d = nc.dram_tensor("woutT", [2 * HD, H], F32R, kind="ExternalInput").ap()
    bqk_d = nc.dram_tensor("bqk", [128, 4], F32, kind="ExternalInput").ap()
    bv_d = nc.dram_tensor("bv", [128, 2 * HD], F32, kind="ExternalInput").ap()
    tab1_d = nc.dram_tensor("tab1", [128, S], F32, kind="ExternalInput").ap()
    tab2_d = nc.dram_tensor("tab2", [128, S], F32, kind="ExternalInput").ap()
    tri_d = nc.dram_tensor("tri", [128, 128], F32, kind="ExternalInput").ap()
    ones_d = nc.dram_tensor("ones", [128, 128], F32R, kind="ExternalInput").ap()
    out_d = nc.dram_tensor("outp", [T, H], F32, kind="ExternalOutput").ap()

    with tile.TileContext(nc) as tc:
        with (
            tc.tile_pool(name="consts", bufs=1) as consts,
            tc.tile_pool(name="qkpre", bufs=1) as qkpre_pool,
            tc.tile_pool(name="vpool", bufs=1) as v_pool,
            tc.tile_pool(name="ytpool", bufs=1) as yt_pool,
            tc.tile_pool(name="xpool", bufs=6) as x_pool,
            tc.tile_pool(name="rope", bufs=4) as rope_pool,
            tc.tile_pool(name="ptpool", bufs=6) as pt_pool,
            tc.tile_pool(name="small", bufs=1) as small_pool,
            tc.tile_pool(name="accpool", bufs=1) as acc_pool,
            tc.tile_pool(name="ps256", bufs=3, space="PSUM") as ps256,
            tc.tile_pool(name="psA", bufs=3, space="PSUM") as psA,
            tc.tile_pool(name="psY", bufs=2, space="PSUM") as psY,
        ):
            # ---- resident constants ----
            wqk_s = consts.tile([128, 16 * 512], F32R, name="wqk_s")

            def load_wqk_j(j):
                nc.sync.dma_start(
                    wqk_s[:].rearrange("p (g o) -> p g o", g=16)[
                        :, :, 128 * j : 128 * (j + 1)],
                    wqk_d.rearrange("(g p) o -> p g o", p=128)[
                        :, :, 128 * j : 128 * (j + 1)],
                )
            wv_s = consts.tile([128, 16 * 256], F32R, name="wv_s")
            bqk_s = consts.tile([128, 4], F32, name="bqk_s")
            bv_s = consts.tile([128, 2 * HD], F32, name="bv_s")
            # loaded later (first needed once chunk 0's matmuls are running)
            # so the startup DMA burst only covers wqk + the first x chunks
            tab1_s = consts.tile([128, S], F32, name="tab1_s")
            tab2_s = consts.tile([128, S], F32, name="tab2_s")
            tri_s = consts.tile([128, 128], F32, name="tri_s")
            ones_s = consts.tile([128, 128], F32R, name="ones_s")
            wo_s = consts.tile([128, 2 * H], F32R, name="wo_s")

            def load_consts_a():
                nc.sync.dma_start(bqk_s[:], bqk_d[:])
                nc.sync.dma_start(
                    wv_s[:].rearrange("p (g o) -> p g o", g=16),
                    wv_d.rearrange("(g p) o -> p g o", p=128),
                )
                nc.sync.dma_start(bv_s[:], bv_d[:])

            def load_consts_b():
                nc.sync.dma_start(tab1_s[:], tab1_d[:])
                nc.sync.dma_start(tab2_s[:], tab2_d[:])

            def load_consts_c():
                nc.sync.dma_start(tri_s[:], tri_d[:])
                nc.sync.dma_start(ones_s[:], ones_d[:])
                nc.sync.dma_start(
                    wo_s[:].rearrange("p (g m) -> p g m", g=2),
                    wout_d.rearrange("(g p) m -> p g m", p=128),
                )

            for b in range(B):
                t0 = b * S

                # j order: q_h0, q_h1, k_h0, k_h1
                pre = [
                    qkpre_pool.tile([128, S], F32R, tag=f"pre{j}", name=f"pre{j}_{b}")
                    for j in range(4)
                ]
                v_s = v_pool.tile([128, 16 * 256], F32R, tag="v", name=f"v_{b}")
                yt = [
                    yt_pool.tile([128, S], F32R, tag=f"yt{h}", name=f"yt{h}_{b}")
                    for h in range(HPC)
                ]

                # ---------------- phase 1: QKV projection ----------------
                def emit_x(c, b=b, t0=t0):
                    tc_off = t0 + CHUNK * c
                    xh = []
                    xT_3d = xT_d.rearrange("(g p) t -> p g t", p=128)
                    for qtr in range(4):
                        xt = x_pool.tile([128, 4 * CHUNK], F32R, tag="xc",
                                         name=f"xc_{b}_{c}_{qtr}")
                        nc.sync.dma_start(
                            xt[:].rearrange("p (g c) -> p g c", g=4),
                            xT_3d[:, 4 * qtr : 4 * (qtr + 1),
                                  tc_off : tc_off + CHUNK],
                        )
                        xh.append(xt)
                    return xh

                def emit_chunk(c, xh, b=b, t0=t0, pre=pre, v_s=v_s):
                    for j in range(4):
                        ps = ps256.tile([128, CHUNK], F32, tag="p1",
                                        name=f"psqk_{b}_{c}_{j}")
                        for ht in range(16):
                            nc.tensor.matmul(
                                ps[:],
                                wqk_s[:, 512 * ht + 128 * j : 512 * ht + 128 * (j + 1)],
                                xh[ht // 4][:, CHUNK * (ht % 4) : CHUNK * (ht % 4 + 1)],
                                start=(ht == 0),
                                stop=(ht == 15),
                            )
                        nc.scalar.activation(
                            pre[j][:, CHUNK * c : CHUNK * (c + 1)],
                            ps[:],
                            AF.Identity,
                            bias=bqk_s[:, j : j + 1],
                        )

                    for tt in range(2):
                        gtt = 2 * c + tt  # t-tile within batch
                        psv = ps256.tile([128, 256], F32, tag="p1",
                                         name=f"psv_{b}_{c}_{tt}")
                        for ht in range(16):
                            nc.tensor.matmul(
                                psv[:],
                                xh[ht // 4][:, CHUNK * (ht % 4) + 128 * tt :
                                            CHUNK * (ht % 4) + 128 * (tt + 1)],
                                wv_s[:, 256 * ht : 256 * (ht + 1)],
                                start=(ht == 0),
                                stop=(ht == 15),
                            )
                        nc.vector.tensor_add(
                            v_s[:, 256 * gtt : 256 * (gtt + 1)], psv[:], bv_s[:]
                        )

                    if b == 0 and c == 0:
                        load_consts_a()
                    if b == 0 and c == 1:
                        load_consts_b()

                    # rope per 512-column group, overlapped with phase 1
                    if c % 2 == 1:
                        cs = slice(CHUNK * (c - 1), CHUNK * (c + 1))
                        for j in range(4):
                            swap = rope_pool.tile([128, 512], F32R, tag="swap",
                                                  name=f"swap_{b}_{c}_{j}")
                            nc.scalar.dma_start(swap[0:64, :], pre[j][64:128, cs])
                            nc.scalar.dma_start(swap[64:128, :], pre[j][0:64, cs])
                            nc.vector.tensor_mul(
                                swap[:], _f(swap[:]), tab2_s[:, cs])
                            nc.vector.tensor_mul(
                                pre[j][:, cs], _f(pre[j][:, cs]), tab1_s[:, cs])
                            nc.vector.tensor_add(
                                pre[j][:, cs], _f(pre[j][:, cs]), _f(swap[:]))

                # ---------------- phase 2: SDPA (interleaved per i-group) ---
                def emit_sdpa(h, ci, b=b, rot=pre, v_s=v_s, yt=yt):
                    qT = rot[h]
                    kT = rot[2 + h]
                    if True:
                        i0 = 512 * ci
                        nj = 4 * ci + 4
                        ps_y = psY.tile([128, 512], F32, tag="y",
                                        name=f"psy_{b}_{h}_{ci}")
                        # denominator partials split across DVE (even jt,
                        # f32r) and GPSIMD (odd jt, f32 - POOL is idle)
                        acc = acc_pool.tile([128, 512], F32R, tag="acc",
                                            name=f"acc_{b}_{h}_{ci}")
                        accB = acc_pool.tile([128, 512], F32R, tag="accB",
                                             name=f"accB_{b}_{h}_{ci}")
                        for jt in range(nj):
                            sub = jt - 4 * ci
                            s0 = max(0, 128 * sub)
                            pt = pt_pool.tile([128, 512], F32R, tag="pt",
                                              name=f"pt_{b}_{h}_{ci}_{jt}")
                            ps_s = psA.tile([128, 512], F32, tag="s",
                                            name=f"pss_{b}_{h}_{ci}_{jt}")
                            nc.tensor.matmul(
                                ps_s[:, s0:512],
                                kT[:, 128 * jt : 128 * (jt + 1)],
                                qT[:, i0 + s0 : i0 + 512],
                                start=True, stop=True,
                            )
                            nc.scalar.activation(
                                pt[:, s0:512], ps_s[:, s0:512], AF.Exp)
                            if sub >= 0:
                                nc.vector.tensor_mul(
                                    pt[:, s0 : s0 + 128],
                                    _f(pt[:, s0 : s0 + 128]),
                                    tri_s[:],
                                )
                            # jt==0 always has s0==0, so start=True initializes
                            # the full 512 columns; later partial-width matmuls
                            # accumulate into their valid subrange only.
                            nc.tensor.matmul(
                                ps_y[:, s0:512],
                                v_s[:, 256 * jt + 128 * h : 256 * jt + 128 * (h + 1)],
                                pt[:, s0:512],
                                start=(jt == 0), stop=(jt == nj - 1),
                            )
                            if jt == 0:
                                nc.vector.tensor_copy(acc[:], _f(pt[:]))
                            elif jt == 1:
                                nc.gpsimd.tensor_copy(
                                    accB[:, s0:512], _f(pt[:, s0:512]))
                            elif jt % 2 == 0:
                                nc.vector.tensor_add(
                                    acc[:, s0:512],
                                    _f(acc[:, s0:512]),
                                    _f(pt[:, s0:512]),
                                )
                            else:
                                nc.gpsimd.tensor_add(
                                    accB[:, s0:512],
                                    _f(accB[:, s0:512]),
                                    _f(pt[:, s0:512]),
                                )
                        # normalize: yt = ps_y * (1/denom) broadcast
                        ps_dd = psA.tile([128, 512], F32, tag="s",
                                         name=f"psdd_{b}_{h}_{ci}")
                        nc.tensor.matmul(
                            ps_dd[0:1, :], ones_s[:, 0:1], acc[:],
                            start=True, stop=False,
                        )
                        sB = 128 if ci == 0 else 0  # accB cols written
                        nc.tensor.matmul(
                            ps_dd[0:1, sB:512], ones_s[:, 0:1], accB[:, sB:512],
                            start=False, stop=True,
                        )
                        rec = small_pool.tile([1, 512], F32R, tag="rec",
                                              name=f"rec_{b}_{h}_{ci}")
                        with nc.allow_low_precision("f32r matmul input"):
                            nc.vector.reciprocal(rec[:], ps_dd[0:1, :])
                        ps_b = psA.tile([128, 512], F32, tag="s",
                                        name=f"psb_{b}_{h}_{ci}")
                        nc.tensor.matmul(
                            ps_b[:], ones_s[0:1, :], rec[:],
                            start=True, stop=True,
                        )
                        ystage = pt_pool.tile([128, 512], F32, tag="pt",
                                              name=f"yst_{b}_{h}_{ci}")
                        nc.scalar.copy(ystage[:], ps_y[:])
                        nc.vector.tensor_mul(
                            yt[h][:, i0 : i0 + 512], ystage[:], ps_b[:]
                        )

                # phase 3 (interleaved): out projection for a tt group
                def emit_outproj(ci, b=b, t0=t0, yt=yt):
                    for tt in range(4 * ci, 4 * ci + 4):
                        for mc in range(4):
                            m0 = 512 * mc
                            ps_o = psA.tile([128, 512], F32, tag="s",
                                            name=f"pso_{b}_{tt}_{mc}")
                            for h in range(HPC):
                                nc.tensor.matmul(
                                    ps_o[:],
                                    yt[h][:, 128 * tt : 128 * (tt + 1)],
                                    wo_s[:, 2048 * h + m0 : 2048 * h + m0 + 512],
                                    start=(h == 0), stop=(h == HPC - 1),
                                )
                            stage = pt_pool.tile([128, 512], F32, tag="pt",
                                                 name=f"st_{b}_{tt}_{mc}")
                            if mc % 2 == 0:
                                nc.scalar.copy(stage[:], ps_o[:])
                            else:
                                nc.vector.tensor_copy(stage[:], ps_o[:])
                            eng = nc.gpsimd if mc % 2 == 0 else nc.scalar
                            eng.dma_start(
                                out_d[t0 + 128 * tt : t0 + 128 * (tt + 1),
                                      m0 : m0 + 512],
                                stage[:],
                            )

                # interleave: two phase-1 chunks + rope group, then both
                # heads' SDPA on the freshly completed i-group, then the
                # out-projection rows that group unlocked
                if b == 0:
                    for j in range(4):
                        load_wqk_j(j)
                xq = {0: emit_x(0), 1: emit_x(1)}
                for ci in range(4):
                    emit_chunk(2 * ci, xq.pop(2 * ci))
                    emit_chunk(2 * ci + 1, xq.pop(2 * ci + 1))
                    if ci < 3:
                        xq[2 * ci + 2] = emit_x(2 * ci + 2)
                        xq[2 * ci + 3] = emit_x(2 * ci + 3)
                    if b == 0 and ci == 0:
                        load_consts_c()
                    for h in range(HPC):
                        emit_sdpa(h, ci)
                    emit_outproj(ci)
    nc.compile()
    return nc


def _host_prep(x, w_qkv, b_qkv, w_out, b_out):
    """Build per-core input maps."""
    x2 = round_f32r(np.ascontiguousarray(x.reshape(T, H).T))  # [H, T]

    # rope tables (positions repeat per batch -> [128, S])
    inv = 10000.0 ** (-(np.arange(64, dtype=np.float64)) / 64.0)
    ang = np.arange(S, dtype=np.float64)[:, None] * inv[None, :]  # [S, 64]
    cos = np.cos(ang).T.astype(np.float32)  # [64, S]
    sin = np.sin(ang).T.astype(np.float32)
    tab1 = np.concatenate([cos, cos], axis=0)            # [128, S]
    tab2 = np.concatenate([-sin, sin], axis=0)           # [128, S]

    tri = np.triu(np.ones((128, 128), np.float32))       # [j, i] valid i>=j

    perm = np.concatenate([np.arange(0, 128, 2), np.arange(1, 128, 2)])
    scale = 1.0 / np.sqrt(HD)

    wq = w_qkv[0 * H : 1 * H].reshape(NH, HD, H)
    wk = w_qkv[1 * H : 2 * H].reshape(NH, HD, H)
    wv = w_qkv[2 * H : 3 * H].reshape(NH, HD, H)
    bq = b_qkv[0 * H : 1 * H].reshape(NH, HD)
    bk = b_qkv[1 * H : 2 * H].reshape(NH, HD)
    bv = b_qkv[2 * H : 3 * H].reshape(NH, HD)

    in_maps = []
    for c in range(NCORES):
        hs = [HPC * c + i for i in range(HPC)]
        cols = []
        bcols = []
        for h in hs:  # q heads (scaled + permuted)
            cols.append(wq[h][perm] * scale)
            bcols.append(bq[h][perm] * scale)
        for h in hs:  # k heads (permuted)
            cols.append(wk[h][perm])
            bcols.append(bk[h][perm])
        wqkT = round_f32r(
            np.ascontiguousarray(np.concatenate(cols, axis=0).T.astype(np.float32))
        )
        bqk = np.stack(bcols, axis=1).astype(np.float32)  # [128, 4]

        wvT = round_f32r(np.ascontiguousarray(
            np.concatenate([wv[h] for h in hs], axis=0).T.astype(np.float32)
        ))  # [H, 256]
        bvb = np.tile(
            np.concatenate([bv[h] for h in hs])[None, :], (128, 1)
        ).astype(np.float32)  # [128, 256]

        woutT = round_f32r(np.ascontiguousarray(
            w_out[:, HD * hs[0] : HD * (hs[-1] + 1)].T.astype(np.float32)
        ))  # [256, H]

        in_maps.append({
            "xT": x2,
            "wqkT": wqkT,
            "wvT": wvT,
            "woutT": woutT,
            "bqk": bqk,
            "bv": bvb,
            "tab1": tab1,
            "tab2": tab2,
            "tri": tri,
            "ones": np.ones((128, 128), np.float32),
        })
    return in_maps


_NC_CACHE = None


def get_module():
    global _NC_CACHE
    if _NC_CACHE is None:
        _NC_CACHE = build_module()
    return _NC_CACHE


def kernel(x, w_qkv, b_qkv, w_out, b_out):
    x = np.asarray(x, dtype=np.float32)
    w_qkv = np.asarray(w_qkv, dtype=np.float32)
    b_qkv = np.asarray(b_qkv, dtype=np.float32)
    w_out = np.asarray(w_out, dtype=np.float32)
    b_out = np.asarray(b_out, dtype=np.float32)

    nc = get_module()
    in_maps = _host_prep(x, w_qkv, b_qkv, w_out, b_out)
    res = run_bass_kernel_spmd(nc, in_maps, core_ids=list(range(NCORES)))
    acc = np.zeros((T, H), np.float64)
    for c in range(NCORES):
        acc += res.results[c]["outp"].astype(np.float64)
    out = (acc + b_out.astype(np.float64)[None, :]).astype(np.float32)
    return out.reshape(B, S, H)


# revision 55
# speedup vs baseline: 1.2319x; 1.0190x over previous
"""Causal self-attention (B=2, S=2048, H=2048, 16 heads, hd=128) on 8 trn2 cores.

Sharding: tensor-parallel over heads (2 heads/core). Each core computes its
heads' QKV projection + RoPE + causal SDPA + a partial out-projection
(row-parallel w_out); the all-reduce over cores is done host-side as the
unshard step (sum of partials + b_out).

Device kernel (identical program on all cores, per-core weight data):
  phase 1: qkv = x @ w^T per head, q/k emitted head-major [d, t] via
           lhsT=w^T tiles, v emitted natural [t, d] via lhsT=x^T tiles.
           float32r matmuls (full PE rate, fp32 storage, 11-bit mantissa).
  rope:    even/odd dims pre-permuted into [evens;odds] rows host-side, so
           RoPE = pre*tab1 + swap(pre)*tab2 with a partition half-swap DMA.
  sdpa:    scores^T blocks [j,128 x i,512] = k^T.T @ q^T, exp on ScalarE,
           causal via N-restricted matmuls + triangular mask multiply,
           y^T accumulated in PSUM (lhsT=v_j), denominator via ones-matmul,
           normalize with reciprocal + K=1 broadcast matmul.
  phase 3: partial out [t,m] += y^T.T @ w_out^T slices, DMA to DRAM.

All matmul inputs are float32r: DRAM-sourced arrays are pre-rounded on the
host (RNE to 11-bit mantissa, low 12 bits zeroed); device-produced tiles are
declared f32r so ACT/DVE round on write; compute reads go through a .bitcast
back to f32.
"""

import sys

for _p in ("/opt/trn_rl_repo",):
    if _p not in sys.path:
        sys.path.append(_p)

import numpy as np

import concourse.bass as bass
import concourse.tile as tile
from concourse import bacc, mybir
from concourse.bass_utils import run_bass_kernel_spmd

B = 2
S = 2048
H = 2048
NH = 16
HD = 128
NCORES = 8
HPC = NH // NCORES          # heads per core = 2
T = B * S                   # 4096 flattened tokens
CHUNK = 256                 # phase-1 t-chunk
NCHUNK = S // CHUNK         # 8 per batch
F32 = mybir.dt.float32
F32R = mybir.dt.float32r
AF = mybir.ActivationFunctionType


def _f(ap):
    """View an f32r AP as plain f32 for compute-engine reads."""
    return ap.bitcast(F32)


def round_f32r(a: np.ndarray) -> np.ndarray:
    """Round fp32 to f32r (RNE to 11 explicit mantissa bits, low 12 bits 0)."""
    u = a.astype(np.float32).view(np.uint32)
    r = u + (0x7FF + ((u >> 12) & 1))
    return (r & np.uint32(0xFFFFF000)).view(np.float32)


def build_module():
    nc = bacc.Bacc("TRN2", target_bir_lowering=False, debug=False)

    xT_d = nc.dram_tensor("xT", [H, T], F32R, kind="ExternalInput").ap()
    wqk_d = nc.dram_tensor("wqkT", [H, 4 * HD], F32R, kind="ExternalInput").ap()
    wv_d = nc.dram_tensor("wvT", [H, 2 * HD], F32R, kind="ExternalInput").ap()
    wout_d = nc.dram_tensor("woutT", [2 * HD, H], F32R, kind="ExternalInput").ap()
    bqk_d = nc.dram_tensor("bqk", [128, 4], F32, kind="ExternalInput").ap()
    bv_d = nc.dram_tensor("bv", [128, 2 * HD], F32, kind="ExternalInput").ap()
    tab1_d = nc.dram_tensor("tab1", [128, S], F32, kind="ExternalInput").ap()
    tab2_d = nc.dram_tensor("tab2", [128, S], F32, kind="ExternalInput").ap()
    tri_d = nc.dram_tensor("tri", [128, 128], F32, kind="ExternalInput").ap()
    ones_d = nc.dram_tensor("ones", [128, 128], F32R, kind="ExternalInput").ap()
    out_d = nc.dram_tensor("outp", [T, H], F32, kind="ExternalOutput").ap()

    with tile.TileContext(nc) as tc:
        with (
            tc.tile_pool(name="consts", bufs=1) as consts,
            tc.tile_pool(name="qkpre", bufs=1) as qkpre_pool,
            tc.tile_pool(name="vpool", bufs=1) as v_pool,
            tc.tile_pool(name="ytpool", bufs=1) as yt_pool,
            tc.tile_pool(name="xpool", bufs=6) as x_pool,
            tc.tile_pool(name="rope", bufs=4) as rope_pool,
            tc.tile_pool(name="ptpool", bufs=6) as pt_pool,
            tc.tile_pool(name="small", bufs=1) as small_pool,
            tc.tile_pool(name="accpool", bufs=1) as acc_pool,
            tc.tile_pool(name="ps256", bufs=4, space="PSUM") as ps256,
            tc.tile_pool(name="psA", bufs=3, space="PSUM") as psA,
            tc.tile_pool(name="psY", bufs=1, space="PSUM") as psY,
        ):
            # ---- resident constants ----
            wqk_s = consts.tile([128, 16 * 512], F32R, name="wqk_s")

            def load_wqk_j(j):
                nc.sync.dma_start(
                    wqk_s[:].rearrange("p (g o) -> p g o", g=16)[
                        :, :, 128 * j : 128 * (j + 1)],
                    wqk_d.rearrange("(g p) o -> p g o", p=128)[
                        :, :, 128 * j : 128 * (j + 1)],
                )
            wv_s = consts.tile([128, 16 * 256], F32R, name="wv_s")
            bqk_s = consts.tile([128, 4], F32, name="bqk_s")
            bv_s = consts.tile([128, 2 * HD], F32, name="bv_s")
            # loaded later (first needed once chunk 0's matmuls are running)
            # so the startup DMA burst only covers wqk + the first x chunks
            tab1_s = consts.tile([128, S], F32, name="tab1_s")
            tab2_s = consts.tile([128, S], F32, name="tab2_s")
            tri_s = consts.tile([128, 128], F32, name="tri_s")
            ones_s = consts.tile([128, 128], F32R, name="ones_s")
            wo_s = consts.tile([128, 2 * H], F32R, name="wo_s")

            def load_consts_a():
                nc.sync.dma_start(bqk_s[:], bqk_d[:])
                nc.sync.dma_start(
                    wv_s[:].rearrange("p (g o) -> p g o", g=16),
                    wv_d.rearrange("(g p) o -> p g o", p=128),
                )
                nc.sync.dma_start(bv_s[:], bv_d[:])

            def load_consts_b():
                nc.sync.dma_start(tab1_s[:], tab1_d[:])
                nc.sync.dma_start(tab2_s[:], tab2_d[:])

            def load_consts_c():
                nc.sync.dma_start(tri_s[:], tri_d[:])
                nc.sync.dma_start(ones_s[:], ones_d[:])
                nc.sync.dma_start(
                    wo_s[:].rearrange("p (g m) -> p g m", g=2),
                    wout_d.rearrange("(g p) m -> p g m", p=128),
                )

            for b in range(B):
                t0 = b * S

                # j order: q_h0, q_h1, k_h0, k_h1
                pre = [
                    qkpre_pool.tile([128, S], F32R, tag=f"pre{j}", name=f"pre{j}_{b}")
                    for j in range(4)
                ]
                v_s = v_pool.tile([128, 16 * 256], F32R, tag="v", name=f"v_{b}")
                yt = [
                    yt_pool.tile([128, S], F32R, tag=f"yt{h}", name=f"yt{h}_{b}")
                    for h in range(HPC)
                ]

                # ---------------- phase 1: QKV projection ----------------
                def emit_x(c, b=b, t0=t0):
                    tc_off = t0 + CHUNK * c
                    xh = []
                    xT_3d = xT_d.rearrange("(g p) t -> p g t", p=128)
                    for qtr in range(4):
                        xt = x_pool.tile([128, 4 * CHUNK], F32R, tag="xc",
                                         name=f"xc_{b}_{c}_{qtr}")
                        nc.sync.dma_start(
                            xt[:].rearrange("p (g c) -> p g c", g=4),
                            xT_3d[:, 4 * qtr : 4 * (qtr + 1),
                                  tc_off : tc_off + CHUNK],
                        )
                        xh.append(xt)
                    return xh

                def emit_chunk(c, xh, b=b, t0=t0, pre=pre, v_s=v_s):
                    for j in range(4):
                        ps = ps256.tile([128, CHUNK], F32, tag="p1",
                                        name=f"psqk_{b}_{c}_{j}")
                        for ht in range(16):
                            nc.tensor.matmul(
                                ps[:],
                                wqk_s[:, 512 * ht + 128 * j : 512 * ht + 128 * (j + 1)],
                                xh[ht // 4][:, CHUNK * (ht % 4) : CHUNK * (ht % 4 + 1)],
                                start=(ht == 0),
                                stop=(ht == 15),
                            )
                        nc.scalar.activation(
                            pre[j][:, CHUNK * c : CHUNK * (c + 1)],
                            ps[:],
                            AF.Identity,
                            bias=bqk_s[:, j : j + 1],
                        )

                    for tt in range(2):
                        gtt = 2 * c + tt  # t-tile within batch
                        psv = ps256.tile([128, 256], F32, tag="p1",
                                         name=f"psv_{b}_{c}_{tt}")
                        for ht in range(16):
                            nc.tensor.matmul(
                                psv[:],
                                xh[ht // 4][:, CHUNK * (ht % 4) + 128 * tt :
                                            CHUNK * (ht % 4) + 128 * (tt + 1)],
                                wv_s[:, 256 * ht : 256 * (ht + 1)],
                                start=(ht == 0),
                                stop=(ht == 15),
                            )
                        nc.vector.tensor_add(
                            v_s[:, 256 * gtt : 256 * (gtt + 1)], psv[:], bv_s[:]
                        )

                    if b == 0 and c == 0:
                        load_consts_a()
                    if b == 0 and c == 1:
                        load_consts_b()

                    # rope per 512-column group, overlapped with phase 1
                    if c % 2 == 1:
                        cs = slice(CHUNK * (c - 1), CHUNK * (c + 1))
                        for j in range(4):
                            swap = rope_pool.tile([128, 512], F32R, tag="swap",
                                                  name=f"swap_{b}_{c}_{j}")
                            nc.scalar.dma_start(swap[0:64, :], pre[j][64:128, cs])
                            nc.scalar.dma_start(swap[64:128, :], pre[j][0:64, cs])
                            nc.vector.tensor_mul(
                                swap[:], _f(swap[:]), tab2_s[:, cs])
                            nc.vector.tensor_mul(
                                pre[j][:, cs], _f(pre[j][:, cs]), tab1_s[:, cs])
                            nc.vector.tensor_add(
                                pre[j][:, cs], _f(pre[j][:, cs]), _f(swap[:]))

                # ---------------- phase 2: SDPA (interleaved per i-group) ---
                def emit_sdpa(h, ci, b=b, rot=pre, v_s=v_s, yt=yt):
                    qT = rot[h]
                    kT = rot[2 + h]
                    if True:
                        i0 = 512 * ci
                        nj = 4 * ci + 4
                        ps_y = psY.tile([128, 512], F32, tag="y",
                                        name=f"psy_{b}_{h}_{ci}")
                        # denominator partials split across DVE (even jt,
                        # f32r) and GPSIMD (odd jt, f32 - POOL is idle)
                        acc = acc_pool.tile([128, 512], F32R, tag="acc",
                                            name=f"acc_{b}_{h}_{ci}")
                        accB = acc_pool.tile([128, 512], F32R, tag="accB",
                                             name=f"accB_{b}_{h}_{ci}")
                        for jt in range(nj):
                            sub = jt - 4 * ci
                            s0 = max(0, 128 * sub)
                            pt = pt_pool.tile([128, 512], F32R, tag="pt",
                                              name=f"pt_{b}_{h}_{ci}_{jt}")
                            ps_s = psA.tile([128, 512], F32, tag="s",
                                            name=f"pss_{b}_{h}_{ci}_{jt}")
                            nc.tensor.matmul(
                                ps_s[:, s0:512],
                                kT[:, 128 * jt : 128 * (jt + 1)],
                                qT[:, i0 + s0 : i0 + 512],
                                start=True, stop=True,
                            )
                            nc.scalar.activation(
                                pt[:, s0:512], ps_s[:, s0:512], AF.Exp)
                            if sub >= 0:
                                nc.vector.tensor_mul(
                                    pt[:, s0 : s0 + 128],
                                    _f(pt[:, s0 : s0 + 128]),
                                    tri_s[:],
                                )
                            # jt==0 always has s0==0, so start=True initializes
                            # the full 512 columns; later partial-width matmuls
                            # accumulate into their valid subrange only.
                            nc.tensor.matmul(
                                ps_y[:, s0:512],
                                v_s[:, 256 * jt + 128 * h : 256 * jt + 128 * (h + 1)],
                                pt[:, s0:512],
                                start=(jt == 0), stop=(jt == nj - 1),
                            )
                            if jt == 0:
                                nc.vector.tensor_copy(acc[:], _f(pt[:]))
                            elif jt == 1:
                                nc.gpsimd.tensor_copy(
                                    accB[:, s0:512], _f(pt[:, s0:512]))
                            elif jt % 2 == 0:
                                nc.vector.tensor_add(
                                    acc[:, s0:512],
                                    _f(acc[:, s0:512]),
                                    _f(pt[:, s0:512]),
                                )
                            else:
                                nc.gpsimd.tensor_add(
                                    accB[:, s0:512],
                                    _f(accB[:, s0:512]),
                                    _f(pt[:, s0:512]),
                                )
                        # normalize: yt = ps_y * (1/denom) broadcast
                        ps_dd = psA.tile([128, 512], F32, tag="s",
                                         name=f"psdd_{b}_{h}_{ci}")
                        nc.tensor.matmul(
                            ps_dd[0:1, :], ones_s[:, 0:1], acc[:],
                            start=True, stop=False,
                        )
                        sB = 128 if ci == 0 else 0  # accB cols written
                        nc.tensor.matmul(
                            ps_dd[0:1, sB:512], ones_s[:, 0:1], accB[:, sB:512],
                            start=False, stop=True,
                        )
                        rec = small_pool.tile([1, 512], F32R, tag="rec",
                                              name=f"rec_{b}_{h}_{ci}")
                        with nc.allow_low_precision("f32r matmul input"):
                            nc.vector.reciprocal(rec[:], ps_dd[0:1, :])
                        ps_b = psA.tile([128, 512], F32, tag="s",
                                        name=f"psb_{b}_{h}_{ci}")
                        nc.tensor.matmul(
                            ps_b[:], ones_s[0:1, :], rec[:],
                            start=True, stop=True,
                        )
                        ystage = pt_pool.tile([128, 512], F32, tag="pt",
                                              name=f"yst_{b}_{h}_{ci}")
                        nc.scalar.copy(ystage[:], ps_y[:])
                        nc.vector.tensor_mul(
                            yt[h][:, i0 : i0 + 512], ystage[:], ps_b[:]
                        )

                # phase 3 (interleaved): out projection for a tt group
                def emit_outproj(ci, b=b, t0=t0, yt=yt):
                    for tt in range(4 * ci, 4 * ci + 4):
                        for mc in range(4):
                            m0 = 512 * mc
                            ps_o = psA.tile([128, 512], F32, tag="s",
                                            name=f"pso_{b}_{tt}_{mc}")
                            for h in range(HPC):
                                nc.tensor.matmul(
                                    ps_o[:],
                                    yt[h][:, 128 * tt : 128 * (tt + 1)],
                                    wo_s[:, 2048 * h + m0 : 2048 * h + m0 + 512],
                                    start=(h == 0), stop=(h == HPC - 1),
                                )
                            stage = pt_pool.tile([128, 512], F32, tag="pt",
                                                 name=f"st_{b}_{tt}_{mc}")
                            if mc % 2 == 0:
                                nc.scalar.copy(stage[:], ps_o[:])
                            else:
                                nc.vector.tensor_copy(stage[:], ps_o[:])
                            eng = nc.gpsimd if mc % 2 == 0 else nc.scalar
                            eng.dma_start(
                                out_d[t0 + 128 * tt : t0 + 128 * (tt + 1),
                                      m0 : m0 + 512],
                                stage[:],
                            )

                # interleave: two phase-1 chunks + rope group, then both
                # heads' SDPA on the freshly completed i-group, then the
                # out-projection rows that group unlocked
                if b == 0:
                    for j in range(4):
                        load_wqk_j(j)
                xq = {0: emit_x(0), 1: emit_x(1)}
                for ci in range(4):
                    emit_chunk(2 * ci, xq.pop(2 * ci))
                    emit_chunk(2 * ci + 1, xq.pop(2 * ci + 1))
                    if ci < 3:
                        xq[2 * ci + 2] = emit_x(2 * ci + 2)
                        xq[2 * ci + 3] = emit_x(2 * ci + 3)
                    if b == 0 and ci == 0:
                        load_consts_c()
                    for h in range(HPC):
                        emit_sdpa(h, ci)
                    emit_outproj(ci)
    nc.compile()
    return nc


def _host_prep(x, w_qkv, b_qkv, w_out, b_out):
    """Build per-core input maps."""
    x2 = round_f32r(np.ascontiguousarray(x.reshape(T, H).T))  # [H, T]

    # rope tables (positions repeat per batch -> [128, S])
    inv = 10000.0 ** (-(np.arange(64, dtype=np.float64)) / 64.0)
    ang = np.arange(S, dtype=np.float64)[:, None] * inv[None, :]  # [S, 64]
    cos = np.cos(ang).T.astype(np.float32)  # [64, S]
    sin = np.sin(ang).T.astype(np.float32)
    tab1 = np.concatenate([cos, cos], axis=0)            # [128, S]
    tab2 = np.concatenate([-sin, sin], axis=0)           # [128, S]

    tri = np.triu(np.ones((128, 128), np.float32))       # [j, i] valid i>=j

    perm = np.concatenate([np.arange(0, 128, 2), np.arange(1, 128, 2)])
    scale = 1.0 / np.sqrt(HD)

    wq = w_qkv[0 * H : 1 * H].reshape(NH, HD, H)
    wk = w_qkv[1 * H : 2 * H].reshape(NH, HD, H)
    wv = w_qkv[2 * H : 3 * H].reshape(NH, HD, H)
    bq = b_qkv[0 * H : 1 * H].reshape(NH, HD)
    bk = b_qkv[1 * H : 2 * H].reshape(NH, HD)
    bv = b_qkv[2 * H : 3 * H].reshape(NH, HD)

    in_maps = []
    for c in range(NCORES):
        hs = [HPC * c + i for i in range(HPC)]
        cols = []
        bcols = []
        for h in hs:  # q heads (scaled + permuted)
            cols.append(wq[h][perm] * scale)
            bcols.append(bq[h][perm] * scale)
        for h in hs:  # k heads (permuted)
            cols.append(wk[h][perm])
            bcols.append(bk[h][perm])
        wqkT = round_f32r(
            np.ascontiguousarray(np.concatenate(cols, axis=0).T.astype(np.float32))
        )
        bqk = np.stack(bcols, axis=1).astype(np.float32)  # [128, 4]

        wvT = round_f32r(np.ascontiguousarray(
            np.concatenate([wv[h] for h in hs], axis=0).T.astype(np.float32)
        ))  # [H, 256]
        bvb = np.tile(
            np.concatenate([bv[h] for h in hs])[None, :], (128, 1)
        ).astype(np.float32)  # [128, 256]

        woutT = round_f32r(np.ascontiguousarray(
            w_out[:, HD * hs[0] : HD * (hs[-1] + 1)].T.astype(np.float32)
        ))  # [256, H]

        in_maps.append({
            "xT": x2,
            "wqkT": wqkT,
            "wvT": wvT,
            "woutT": woutT,
            "bqk": bqk,
            "bv": bvb,
            "tab1": tab1,
            "tab2": tab2,
            "tri": tri,
            "ones": np.ones((128, 128), np.float32),
        })
    return in_maps


_NC_CACHE = None


def get_module():
    global _NC_CACHE
    if _NC_CACHE is None:
        _NC_CACHE = build_module()
    return _NC_CACHE


def kernel(x, w_qkv, b_qkv, w_out, b_out):
    x = np.asarray(x, dtype=np.float32)
    w_qkv = np.asarray(w_qkv, dtype=np.float32)
    b_qkv = np.asarray(b_qkv, dtype=np.float32)
    w_out = np.asarray(w_out, dtype=np.float32)
    b_out = np.asarray(b_out, dtype=np.float32)

    nc = get_module()
    in_maps = _host_prep(x, w_qkv, b_qkv, w_out, b_out)
    res = run_bass_kernel_spmd(nc, in_maps, core_ids=list(range(NCORES)))
    acc = np.zeros((T, H), np.float64)
    for c in range(NCORES):
        acc += res.results[c]["outp"].astype(np.float64)
    out = (acc + b_out.astype(np.float64)[None, :]).astype(np.float32)
    return out.reshape(B, S, H)


# revision 58
# speedup vs baseline: 1.2548x; 1.0186x over previous
"""Causal self-attention (B=2, S=2048, H=2048, 16 heads, hd=128) on 8 trn2 cores.

Sharding: tensor-parallel over heads (2 heads/core). Each core computes its
heads' QKV projection + RoPE + causal SDPA + a partial out-projection
(row-parallel w_out); the all-reduce over cores is done host-side as the
unshard step (sum of partials + b_out).

Device kernel (identical program on all cores, per-core weight data):
  phase 1: qkv = x @ w^T per head, q/k emitted head-major [d, t] via
           lhsT=w^T tiles, v emitted natural [t, d] via lhsT=x^T tiles.
           float32r matmuls (full PE rate, fp32 storage, 11-bit mantissa).
  rope:    even/odd dims pre-permuted into [evens;odds] rows host-side, so
           RoPE = pre*tab1 + swap(pre)*tab2 with a partition half-swap DMA.
  sdpa:    scores^T blocks [j,128 x i,512] = k^T.T @ q^T, exp on ScalarE,
           causal via N-restricted matmuls + triangular mask multiply,
           y^T accumulated in PSUM (lhsT=v_j), denominator via ones-matmul,
           normalize with reciprocal + K=1 broadcast matmul.
  phase 3: partial out [t,m] += y^T.T @ w_out^T slices, DMA to DRAM.

All matmul inputs are float32r: DRAM-sourced arrays are pre-rounded on the
host (RNE to 11-bit mantissa, low 12 bits zeroed); device-produced tiles are
declared f32r so ACT/DVE round on write; compute reads go through a .bitcast
back to f32.
"""

import sys

for _p in ("/opt/trn_rl_repo",):
    if _p not in sys.path:
        sys.path.append(_p)

import numpy as np

import concourse.bass as bass
import concourse.tile as tile
from concourse import bacc, mybir
from concourse.bass_utils import run_bass_kernel_spmd

B = 2
S = 2048
H = 2048
NH = 16
HD = 128
NCORES = 8
HPC = NH // NCORES          # heads per core = 2
T = B * S                   # 4096 flattened tokens
CHUNK = 256                 # phase-1 t-chunk
NCHUNK = S // CHUNK         # 8 per batch
F32 = mybir.dt.float32
F32R = mybir.dt.float32r
AF = mybir.ActivationFunctionType


def _f(ap):
    """View an f32r AP as plain f32 for compute-engine reads."""
    return ap.bitcast(F32)


def round_f32r(a: np.ndarray) -> np.ndarray:
    """Round fp32 to f32r (RNE to 11 explicit mantissa bits, low 12 bits 0)."""
    u = a.astype(np.float32).view(np.uint32)
    r = u + (0x7FF + ((u >> 12) & 1))
    return (r & np.uint32(0xFFFFF000)).view(np.float32)


def build_module():
    nc = bacc.Bacc("TRN2", target_bir_lowering=False, debug=False)

    xT_d = nc.dram_tensor("xT", [H, T], F32R, kind="ExternalInput").ap()
    wqk_d = nc.dram_tensor("wqkT", [H, 4 * HD], F32R, kind="ExternalInput").ap()
    wv_d = nc.dram_tensor("wvT", [H, 2 * HD], F32R, kind="ExternalInput").ap()
    wout_d = nc.dram_tensor("woutT", [2 * HD, H], F32R, kind="ExternalInput").ap()
    bqk_d = nc.dram_tensor("bqk", [128, 4], F32, kind="ExternalInput").ap()
    bv_d = nc.dram_tensor("bv", [128, 2 * HD], F32, kind="ExternalInput").ap()
    tab1_d = nc.dram_tensor("tab1", [128, S], F32, kind="ExternalInput").ap()
    tab2_d = nc.dram_tensor("tab2", [128, S], F32, kind="ExternalInput").ap()
    tri_d = nc.dram_tensor("tri", [128, 128], F32, kind="ExternalInput").ap()
    ones_d = nc.dram_tensor("ones", [128, 128], F32R, kind="ExternalInput").ap()
    out_d = nc.dram_tensor("outp", [T, H], F32, kind="ExternalOutput").ap()

    with tile.TileContext(nc) as tc:
        with (
            tc.tile_pool(name="consts", bufs=1) as consts,
            tc.tile_pool(name="qkpre", bufs=1) as qkpre_pool,
            tc.tile_pool(name="vpool", bufs=1) as v_pool,
            tc.tile_pool(name="ytpool", bufs=1) as yt_pool,
            tc.tile_pool(name="xpool", bufs=6) as x_pool,
            tc.tile_pool(name="rope", bufs=4) as rope_pool,
            tc.tile_pool(name="ptpool", bufs=6) as pt_pool,
            tc.tile_pool(name="small", bufs=1) as small_pool,
            tc.tile_pool(name="accpool", bufs=1) as acc_pool,
            tc.tile_pool(name="ps256", bufs=4, space="PSUM") as ps256,
            tc.tile_pool(name="psA", bufs=3, space="PSUM") as psA,
            tc.tile_pool(name="psY", bufs=1, space="PSUM") as psY,
        ):
            # ---- resident constants ----
            wqk_s = consts.tile([128, 16 * 512], F32R, name="wqk_s")

            def load_wqk_j(j):
                nc.sync.dma_start(
                    wqk_s[:].rearrange("p (g o) -> p g o", g=16)[
                        :, :, 128 * j : 128 * (j + 1)],
                    wqk_d.rearrange("(g p) o -> p g o", p=128)[
                        :, :, 128 * j : 128 * (j + 1)],
                )
            wv_s = consts.tile([128, 16 * 256], F32R, name="wv_s")
            bqk_s = consts.tile([128, 4], F32, name="bqk_s")
            bv_s = consts.tile([128, 2 * HD], F32, name="bv_s")
            # loaded later (first needed once chunk 0's matmuls are running)
            # so the startup DMA burst only covers wqk + the first x chunks
            tab1_s = consts.tile([128, S], F32, name="tab1_s")
            tab2_s = consts.tile([128, S], F32, name="tab2_s")
            tri_s = consts.tile([128, 128], F32, name="tri_s")
            ones_s = consts.tile([128, 128], F32R, name="ones_s")
            wo_s = consts.tile([128, 2 * H], F32R, name="wo_s")

            def load_consts_a():
                nc.sync.dma_start(
                    wv_s[:].rearrange("p (g o) -> p g o", g=16),
                    wv_d.rearrange("(g p) o -> p g o", p=128),
                )
                nc.sync.dma_start(bv_s[:], bv_d[:])

            def load_consts_b():
                nc.sync.dma_start(tab1_s[:], tab1_d[:])
                nc.sync.dma_start(tab2_s[:], tab2_d[:])

            def load_consts_c():
                nc.sync.dma_start(tri_s[:], tri_d[:])
                nc.sync.dma_start(ones_s[:], ones_d[:])

            def load_wo():
                nc.sync.dma_start(
                    wo_s[:].rearrange("p (g m) -> p g m", g=2),
                    wout_d.rearrange("(g p) m -> p g m", p=128),
                )

            for b in range(B):
                t0 = b * S

                # j order: q_h0, q_h1, k_h0, k_h1
                pre = [
                    qkpre_pool.tile([128, S], F32R, tag=f"pre{j}", name=f"pre{j}_{b}")
                    for j in range(4)
                ]
                v_s = v_pool.tile([128, 16 * 256], F32R, tag="v", name=f"v_{b}")
                yt = [
                    yt_pool.tile([128, S], F32R, tag=f"yt{h}", name=f"yt{h}_{b}")
                    for h in range(HPC)
                ]

                # ---------------- phase 1: QKV projection ----------------
                def emit_x(c, b=b, t0=t0):
                    tc_off = t0 + CHUNK * c
                    xh = []
                    xT_3d = xT_d.rearrange("(g p) t -> p g t", p=128)
                    for qtr in range(4):
                        xt = x_pool.tile([128, 4 * CHUNK], F32R, tag="xc",
                                         name=f"xc_{b}_{c}_{qtr}")
                        nc.sync.dma_start(
                            xt[:].rearrange("p (g c) -> p g c", g=4),
                            xT_3d[:, 4 * qtr : 4 * (qtr + 1),
                                  tc_off : tc_off + CHUNK],
                        )
                        xh.append(xt)
                    return xh

                def emit_chunk(c, xh, b=b, t0=t0, pre=pre, v_s=v_s):
                    for j in range(4):
                        ps = ps256.tile([128, CHUNK], F32, tag="p1",
                                        name=f"psqk_{b}_{c}_{j}")
                        for ht in range(16):
                            nc.tensor.matmul(
                                ps[:],
                                wqk_s[:, 512 * ht + 128 * j : 512 * ht + 128 * (j + 1)],
                                xh[ht // 4][:, CHUNK * (ht % 4) : CHUNK * (ht % 4 + 1)],
                                start=(ht == 0),
                                stop=(ht == 15),
                            )
                        nc.scalar.activation(
                            pre[j][:, CHUNK * c : CHUNK * (c + 1)],
                            ps[:],
                            AF.Identity,
                            bias=bqk_s[:, j : j + 1],
                        )

                    for tt in range(2):
                        gtt = 2 * c + tt  # t-tile within batch
                        psv = ps256.tile([128, 256], F32, tag="p1",
                                         name=f"psv_{b}_{c}_{tt}")
                        for ht in range(16):
                            nc.tensor.matmul(
                                psv[:],
                                xh[ht // 4][:, CHUNK * (ht % 4) + 128 * tt :
                                            CHUNK * (ht % 4) + 128 * (tt + 1)],
                                wv_s[:, 256 * ht : 256 * (ht + 1)],
                                start=(ht == 0),
                                stop=(ht == 15),
                            )
                        nc.vector.tensor_add(
                            v_s[:, 256 * gtt : 256 * (gtt + 1)], psv[:], bv_s[:]
                        )

                    if b == 0 and c == 0:
                        load_consts_a()
                    if b == 0 and c == 1:
                        load_consts_b()

                    # rope per 512-column group, overlapped with phase 1
                    if c % 2 == 1:
                        cs = slice(CHUNK * (c - 1), CHUNK * (c + 1))
                        for j in range(4):
                            swap = rope_pool.tile([128, 512], F32R, tag="swap",
                                                  name=f"swap_{b}_{c}_{j}")
                            nc.scalar.dma_start(swap[0:64, :], pre[j][64:128, cs])
                            nc.scalar.dma_start(swap[64:128, :], pre[j][0:64, cs])
                            nc.vector.tensor_mul(
                                swap[:], _f(swap[:]), tab2_s[:, cs])
                            nc.vector.tensor_mul(
                                pre[j][:, cs], _f(pre[j][:, cs]), tab1_s[:, cs])
                            nc.vector.tensor_add(
                                pre[j][:, cs], _f(pre[j][:, cs]), _f(swap[:]))

                # ---------------- phase 2: SDPA (interleaved per i-group) ---
                def emit_sdpa(h, ci, b=b, rot=pre, v_s=v_s, yt=yt):
                    qT = rot[h]
                    kT = rot[2 + h]
                    if True:
                        i0 = 512 * ci
                        nj = 4 * ci + 4
                        ps_y = psY.tile([128, 512], F32, tag="y",
                                        name=f"psy_{b}_{h}_{ci}")
                        # denominator partials split across DVE (even jt,
                        # f32r) and GPSIMD (odd jt, f32 - POOL is idle)
                        acc = acc_pool.tile([128, 512], F32R, tag="acc",
                                            name=f"acc_{b}_{h}_{ci}")
                        accB = acc_pool.tile([128, 512], F32R, tag="accB",
                                             name=f"accB_{b}_{h}_{ci}")
                        for jt in range(nj):
                            sub = jt - 4 * ci
                            s0 = max(0, 128 * sub)
                            pt = pt_pool.tile([128, 512], F32R, tag="pt",
                                              name=f"pt_{b}_{h}_{ci}_{jt}")
                            ps_s = psA.tile([128, 512], F32, tag="s",
                                            name=f"pss_{b}_{h}_{ci}_{jt}")
                            nc.tensor.matmul(
                                ps_s[:, s0:512],
                                kT[:, 128 * jt : 128 * (jt + 1)],
                                qT[:, i0 + s0 : i0 + 512],
                                start=True, stop=True,
                            )
                            nc.scalar.activation(
                                pt[:, s0:512], ps_s[:, s0:512], AF.Exp)
                            if sub >= 0:
                                nc.vector.tensor_mul(
                                    pt[:, s0 : s0 + 128],
                                    _f(pt[:, s0 : s0 + 128]),
                                    tri_s[:],
                                )
                            # jt==0 always has s0==0, so start=True initializes
                            # the full 512 columns; later partial-width matmuls
                            # accumulate into their valid subrange only.
                            nc.tensor.matmul(
                                ps_y[:, s0:512],
                                v_s[:, 256 * jt + 128 * h : 256 * jt + 128 * (h + 1)],
                                pt[:, s0:512],
                                start=(jt == 0), stop=(jt == nj - 1),
                            )
                            if jt == 0:
                                nc.vector.tensor_copy(acc[:], _f(pt[:]))
                            elif jt == 1:
                                nc.gpsimd.tensor_copy(
                                    accB[:, s0:512], _f(pt[:, s0:512]))
                            elif jt % 2 == 0:
                                nc.vector.tensor_add(
                                    acc[:, s0:512],
                                    _f(acc[:, s0:512]),
                                    _f(pt[:, s0:512]),
                                )
                            else:
                                nc.gpsimd.tensor_add(
                                    accB[:, s0:512],
                                    _f(accB[:, s0:512]),
                                    _f(pt[:, s0:512]),
                                )
                        # normalize: yt = ps_y * (1/denom) broadcast
                        ps_dd = psA.tile([128, 512], F32, tag="s",
                                         name=f"psdd_{b}_{h}_{ci}")
                        nc.tensor.matmul(
                            ps_dd[0:1, :], ones_s[:, 0:1], acc[:],
                            start=True, stop=False,
                        )
                        sB = 128 if ci == 0 else 0  # accB cols written
                        nc.tensor.matmul(
                            ps_dd[0:1, sB:512], ones_s[:, 0:1], accB[:, sB:512],
                            start=False, stop=True,
                        )
                        rec = small_pool.tile([1, 512], F32R, tag="rec",
                                              name=f"rec_{b}_{h}_{ci}")
                        with nc.allow_low_precision("f32r matmul input"):
                            nc.vector.reciprocal(rec[:], ps_dd[0:1, :])
                        ps_b = psA.tile([128, 512], F32, tag="s",
                                        name=f"psb_{b}_{h}_{ci}")
                        nc.tensor.matmul(
                            ps_b[:], ones_s[0:1, :], rec[:],
                            start=True, stop=True,
                        )
                        ystage = pt_pool.tile([128, 512], F32, tag="pt",
                                              name=f"yst_{b}_{h}_{ci}")
                        nc.scalar.copy(ystage[:], ps_y[:])
                        nc.vector.tensor_mul(
                            yt[h][:, i0 : i0 + 512], ystage[:], ps_b[:]
                        )

                # phase 3 (interleaved): out projection for a tt group
                def emit_outproj(ci, b=b, t0=t0, yt=yt):
                    for tt in range(4 * ci, 4 * ci + 4):
                        for mc in range(4):
                            m0 = 512 * mc
                            ps_o = psA.tile([128, 512], F32, tag="s",
                                            name=f"pso_{b}_{tt}_{mc}")
                            for h in range(HPC):
                                nc.tensor.matmul(
                                    ps_o[:],
                                    yt[h][:, 128 * tt : 128 * (tt + 1)],
                                    wo_s[:, 2048 * h + m0 : 2048 * h + m0 + 512],
                                    start=(h == 0), stop=(h == HPC - 1),
                                )
                            stage = pt_pool.tile([128, 512], F32, tag="pt",
                                                 name=f"st_{b}_{tt}_{mc}")
                            if mc % 2 == 0:
                                nc.scalar.copy(stage[:], ps_o[:])
                            else:
                                nc.vector.tensor_copy(stage[:], ps_o[:])
                            eng = nc.gpsimd if mc % 2 == 0 else nc.scalar
                            eng.dma_start(
                                out_d[t0 + 128 * tt : t0 + 128 * (tt + 1),
                                      m0 : m0 + 512],
                                stage[:],
                            )

                # interleave: two phase-1 chunks + rope group, then both
                # heads' SDPA on the freshly completed i-group, then the
                # out-projection rows that group unlocked
                if b == 0:
                    load_wqk_j(0)
                    nc.sync.dma_start(bqk_s[:], bqk_d[:])
                    xq = {0: emit_x(0)}
                    load_wqk_j(1)
                    load_wqk_j(2)
                    load_wqk_j(3)
                    xq[1] = emit_x(1)
                else:
                    xq = {0: emit_x(0), 1: emit_x(1)}
                for ci in range(4):
                    emit_chunk(2 * ci, xq.pop(2 * ci))
                    emit_chunk(2 * ci + 1, xq.pop(2 * ci + 1))
                    if ci < 3:
                        xq[2 * ci + 2] = emit_x(2 * ci + 2)
                        xq[2 * ci + 3] = emit_x(2 * ci + 3)
                    if b == 0 and ci == 0:
                        load_consts_c()
                    for h in range(HPC):
                        emit_sdpa(h, ci)
                    if b == 0 and ci == 0:
                        load_wo()
                    emit_outproj(ci)
    nc.compile()
    return nc


def _host_prep(x, w_qkv, b_qkv, w_out, b_out):
    """Build per-core input maps."""
    x2 = round_f32r(np.ascontiguousarray(x.reshape(T, H).T))  # [H, T]

    # rope tables (positions repeat per batch -> [128, S])
    inv = 10000.0 ** (-(np.arange(64, dtype=np.float64)) / 64.0)
    ang = np.arange(S, dtype=np.float64)[:, None] * inv[None, :]  # [S, 64]
    cos = np.cos(ang).T.astype(np.float32)  # [64, S]
    sin = np.sin(ang).T.astype(np.float32)
    tab1 = np.concatenate([cos, cos], axis=0)            # [128, S]
    tab2 = np.concatenate([-sin, sin], axis=0)           # [128, S]

    tri = np.triu(np.ones((128, 128), np.float32))       # [j, i] valid i>=j

    perm = np.concatenate([np.arange(0, 128, 2), np.arange(1, 128, 2)])
    scale = 1.0 / np.sqrt(HD)

    wq = w_qkv[0 * H : 1 * H].reshape(NH, HD, H)
    wk = w_qkv[1 * H : 2 * H].reshape(NH, HD, H)
    wv = w_qkv[2 * H : 3 * H].reshape(NH, HD, H)
    bq = b_qkv[0 * H : 1 * H].reshape(NH, HD)
    bk = b_qkv[1 * H : 2 * H].reshape(NH, HD)
    bv = b_qkv[2 * H : 3 * H].reshape(NH, HD)

    in_maps = []
    for c in range(NCORES):
        hs = [HPC * c + i for i in range(HPC)]
        cols = []
        bcols = []
        for h in hs:  # q heads (scaled + permuted)
            cols.append(wq[h][perm] * scale)
            bcols.append(bq[h][perm] * scale)
        for h in hs:  # k heads (permuted)
            cols.append(wk[h][perm])
            bcols.append(bk[h][perm])
        wqkT = round_f32r(
            np.ascontiguousarray(np.concatenate(cols, axis=0).T.astype(np.float32))
        )
        bqk = np.stack(bcols, axis=1).astype(np.float32)  # [128, 4]

        wvT = round_f32r(np.ascontiguousarray(
            np.concatenate([wv[h] for h in hs], axis=0).T.astype(np.float32)
        ))  # [H, 256]
        bvb = np.tile(
            np.concatenate([bv[h] for h in hs])[None, :], (128, 1)
        ).astype(np.float32)  # [128, 256]

        woutT = round_f32r(np.ascontiguousarray(
            w_out[:, HD * hs[0] : HD * (hs[-1] + 1)].T.astype(np.float32)
        ))  # [256, H]

        in_maps.append({
            "xT": x2,
            "wqkT": wqkT,
            "wvT": wvT,
            "woutT": woutT,
            "bqk": bqk,
            "bv": bvb,
            "tab1": tab1,
            "tab2": tab2,
            "tri": tri,
            "ones": np.ones((128, 128), np.float32),
        })
    return in_maps


_NC_CACHE = None


def get_module():
    global _NC_CACHE
    if _NC_CACHE is None:
        _NC_CACHE = build_module()
    return _NC_CACHE


def kernel(x, w_qkv, b_qkv, w_out, b_out):
    x = np.asarray(x, dtype=np.float32)
    w_qkv = np.asarray(w_qkv, dtype=np.float32)
    b_qkv = np.asarray(b_qkv, dtype=np.float32)
    w_out = np.asarray(w_out, dtype=np.float32)
    b_out = np.asarray(b_out, dtype=np.float32)

    nc = get_module()
    in_maps = _host_prep(x, w_qkv, b_qkv, w_out, b_out)
    res = run_bass_kernel_spmd(nc, in_maps, core_ids=list(range(NCORES)))
    acc = np.zeros((T, H), np.float64)
    for c in range(NCORES):
        acc += res.results[c]["outp"].astype(np.float64)
    out = (acc + b_out.astype(np.float64)[None, :]).astype(np.float32)
    return out.reshape(B, S, H)


# revision 59
# speedup vs baseline: 1.2957x; 1.0326x over previous
"""Causal self-attention (B=2, S=2048, H=2048, 16 heads, hd=128) on 8 trn2 cores.

Sharding: tensor-parallel over heads (2 heads/core). Each core computes its
heads' QKV projection + RoPE + causal SDPA + a partial out-projection
(row-parallel w_out); the all-reduce over cores is done host-side as the
unshard step (sum of partials + b_out).

Device kernel (identical program on all cores, per-core weight data):
  phase 1: qkv = x @ w^T per head, q/k emitted head-major [d, t] via
           lhsT=w^T tiles, v emitted natural [t, d] via lhsT=x^T tiles.
           float32r matmuls (full PE rate, fp32 storage, 11-bit mantissa).
  rope:    even/odd dims pre-permuted into [evens;odds] rows host-side, so
           RoPE = pre*tab1 + swap(pre)*tab2 with a partition half-swap DMA.
  sdpa:    scores^T blocks [j,128 x i,512] = k^T.T @ q^T, exp on ScalarE,
           causal via N-restricted matmuls + triangular mask multiply,
           y^T accumulated in PSUM (lhsT=v_j), denominator via ones-matmul,
           normalize with reciprocal + K=1 broadcast matmul.
  phase 3: partial out [t,m] += y^T.T @ w_out^T slices, DMA to DRAM.

All matmul inputs are float32r: DRAM-sourced arrays are pre-rounded on the
host (RNE to 11-bit mantissa, low 12 bits zeroed); device-produced tiles are
declared f32r so ACT/DVE round on write; compute reads go through a .bitcast
back to f32.
"""

import sys

for _p in ("/opt/trn_rl_repo",):
    if _p not in sys.path:
        sys.path.append(_p)

import numpy as np

import concourse.bass as bass
import concourse.tile as tile
from concourse import bacc, mybir
from concourse.bass_utils import run_bass_kernel_spmd

B = 2
S = 2048
H = 2048
NH = 16
HD = 128
NCORES = 8
HPC = NH // NCORES          # heads per core = 2
T = B * S                   # 4096 flattened tokens
CHUNK = 512                 # phase-1 t-chunk (= one SDPA i-group)
NCHUNK = S // CHUNK         # 4 per batch
F32 = mybir.dt.float32
F32R = mybir.dt.float32r
AF = mybir.ActivationFunctionType


def _f(ap):
    """View an f32r AP as plain f32 for compute-engine reads."""
    return ap.bitcast(F32)


def round_f32r(a: np.ndarray) -> np.ndarray:
    """Round fp32 to f32r (RNE to 11 explicit mantissa bits, low 12 bits 0)."""
    u = a.astype(np.float32).view(np.uint32)
    r = u + (0x7FF + ((u >> 12) & 1))
    return (r & np.uint32(0xFFFFF000)).view(np.float32)


def build_module():
    nc = bacc.Bacc("TRN2", target_bir_lowering=False, debug=False)

    xT_d = nc.dram_tensor("xT", [H, T], F32R, kind="ExternalInput").ap()
    wqk_d = nc.dram_tensor("wqkT", [H, 4 * HD], F32R, kind="ExternalInput").ap()
    wv_d = nc.dram_tensor("wvT", [H, 2 * HD], F32R, kind="ExternalInput").ap()
    wout_d = nc.dram_tensor("woutT", [2 * HD, H], F32R, kind="ExternalInput").ap()
    bqk_d = nc.dram_tensor("bqk", [128, 4], F32, kind="ExternalInput").ap()
    bv_d = nc.dram_tensor("bv", [128, 2 * HD], F32, kind="ExternalInput").ap()
    tab1_d = nc.dram_tensor("tab1", [128, S], F32, kind="ExternalInput").ap()
    tab2_d = nc.dram_tensor("tab2", [128, S], F32, kind="ExternalInput").ap()
    tri_d = nc.dram_tensor("tri", [128, 128], F32, kind="ExternalInput").ap()
    ones_d = nc.dram_tensor("ones", [128, 128], F32R, kind="ExternalInput").ap()
    out_d = nc.dram_tensor("outp", [T, H], F32, kind="ExternalOutput").ap()

    with tile.TileContext(nc) as tc:
        with (
            tc.tile_pool(name="consts", bufs=1) as consts,
            tc.tile_pool(name="qkpre", bufs=1) as qkpre_pool,
            tc.tile_pool(name="vpool", bufs=1) as v_pool,
            tc.tile_pool(name="ytpool", bufs=1) as yt_pool,
            tc.tile_pool(name="xpool", bufs=8) as x_pool,
            tc.tile_pool(name="rope", bufs=4) as rope_pool,
            tc.tile_pool(name="ptpool", bufs=6) as pt_pool,
            tc.tile_pool(name="small", bufs=1) as small_pool,
            tc.tile_pool(name="accpool", bufs=1) as acc_pool,
            tc.tile_pool(name="ps256", bufs=4, space="PSUM") as ps256,
            tc.tile_pool(name="psA", bufs=3, space="PSUM") as psA,
            tc.tile_pool(name="psY", bufs=1, space="PSUM") as psY,
        ):
            # ---- resident constants ----
            wqk_s = consts.tile([128, 16 * 512], F32R, name="wqk_s")

            def load_wqk_j(j):
                nc.sync.dma_start(
                    wqk_s[:].rearrange("p (g o) -> p g o", g=16)[
                        :, :, 128 * j : 128 * (j + 1)],
                    wqk_d.rearrange("(g p) o -> p g o", p=128)[
                        :, :, 128 * j : 128 * (j + 1)],
                )
            wv_s = consts.tile([128, 16 * 256], F32R, name="wv_s")
            bqk_s = consts.tile([128, 4], F32, name="bqk_s")
            bv_s = consts.tile([128, 2 * HD], F32, name="bv_s")
            # loaded later (first needed once chunk 0's matmuls are running)
            # so the startup DMA burst only covers wqk + the first x chunks
            tab1_s = consts.tile([128, S], F32, name="tab1_s")
            tab2_s = consts.tile([128, S], F32, name="tab2_s")
            tri_s = consts.tile([128, 128], F32, name="tri_s")
            ones_s = consts.tile([128, 128], F32R, name="ones_s")
            wo_s = consts.tile([128, 2 * H], F32R, name="wo_s")

            def load_consts_a():
                nc.sync.dma_start(
                    wv_s[:].rearrange("p (g o) -> p g o", g=16),
                    wv_d.rearrange("(g p) o -> p g o", p=128),
                )
                nc.sync.dma_start(bv_s[:], bv_d[:])

            def load_consts_b():
                nc.sync.dma_start(tab1_s[:], tab1_d[:])
                nc.sync.dma_start(tab2_s[:], tab2_d[:])

            def load_consts_c():
                nc.sync.dma_start(tri_s[:], tri_d[:])
                nc.sync.dma_start(ones_s[:], ones_d[:])

            def load_wo():
                nc.sync.dma_start(
                    wo_s[:].rearrange("p (g m) -> p g m", g=2),
                    wout_d.rearrange("(g p) m -> p g m", p=128),
                )

            for b in range(B):
                t0 = b * S

                # j order: q_h0, q_h1, k_h0, k_h1
                pre = [
                    qkpre_pool.tile([128, S], F32R, tag=f"pre{j}", name=f"pre{j}_{b}")
                    for j in range(4)
                ]
                v_s = v_pool.tile([128, 16 * 256], F32R, tag="v", name=f"v_{b}")
                yt = [
                    yt_pool.tile([128, S], F32R, tag=f"yt{h}", name=f"yt{h}_{b}")
                    for h in range(HPC)
                ]

                # ---------------- phase 1: QKV projection ----------------
                def emit_x(c, b=b, t0=t0):
                    tc_off = t0 + CHUNK * c
                    xh = []
                    xT_3d = xT_d.rearrange("(g p) t -> p g t", p=128)
                    for e in range(8):
                        xt = x_pool.tile([128, 2 * CHUNK], F32R, tag="xc",
                                         name=f"xc_{b}_{c}_{e}")
                        nc.sync.dma_start(
                            xt[:].rearrange("p (g c) -> p g c", g=2),
                            xT_3d[:, 2 * e : 2 * (e + 1),
                                  tc_off : tc_off + CHUNK],
                        )
                        xh.append(xt)
                    return xh

                def emit_chunk(c, xh, b=b, t0=t0, pre=pre, v_s=v_s):
                    for j in range(4):
                        ps = ps256.tile([128, CHUNK], F32, tag="p1",
                                        name=f"psqk_{b}_{c}_{j}")
                        for ht in range(16):
                            nc.tensor.matmul(
                                ps[:],
                                wqk_s[:, 512 * ht + 128 * j : 512 * ht + 128 * (j + 1)],
                                xh[ht // 2][:, CHUNK * (ht % 2) : CHUNK * (ht % 2 + 1)],
                                start=(ht == 0),
                                stop=(ht == 15),
                            )
                        nc.scalar.activation(
                            pre[j][:, CHUNK * c : CHUNK * (c + 1)],
                            ps[:],
                            AF.Identity,
                            bias=bqk_s[:, j : j + 1],
                        )

                    for tt in range(4):
                        gtt = 4 * c + tt  # t-tile within batch
                        psv = ps256.tile([128, 256], F32, tag="p1",
                                         name=f"psv_{b}_{c}_{tt}")
                        for ht in range(16):
                            nc.tensor.matmul(
                                psv[:],
                                xh[ht // 2][:, CHUNK * (ht % 2) + 128 * tt :
                                            CHUNK * (ht % 2) + 128 * (tt + 1)],
                                wv_s[:, 256 * ht : 256 * (ht + 1)],
                                start=(ht == 0),
                                stop=(ht == 15),
                            )
                        nc.vector.tensor_add(
                            v_s[:, 256 * gtt : 256 * (gtt + 1)], psv[:], bv_s[:]
                        )

                    if b == 0 and c == 0:
                        load_consts_a()
                    if b == 0 and c == 0:
                        load_consts_b()

                    # rope for this 512-column chunk
                    if True:
                        cs = slice(CHUNK * c, CHUNK * (c + 1))
                        for j in range(4):
                            swap = rope_pool.tile([128, 512], F32R, tag="swap",
                                                  name=f"swap_{b}_{c}_{j}")
                            nc.scalar.dma_start(swap[0:64, :], pre[j][64:128, cs])
                            nc.scalar.dma_start(swap[64:128, :], pre[j][0:64, cs])
                            nc.vector.tensor_mul(
                                swap[:], _f(swap[:]), tab2_s[:, cs])
                            nc.vector.tensor_mul(
                                pre[j][:, cs], _f(pre[j][:, cs]), tab1_s[:, cs])
                            nc.vector.tensor_add(
                                pre[j][:, cs], _f(pre[j][:, cs]), _f(swap[:]))

                # ---------------- phase 2: SDPA (interleaved per i-group) ---
                def emit_sdpa(h, ci, b=b, rot=pre, v_s=v_s, yt=yt):
                    qT = rot[h]
                    kT = rot[2 + h]
                    if True:
                        i0 = 512 * ci
                        nj = 4 * ci + 4
                        ps_y = psY.tile([128, 512], F32, tag="y",
                                        name=f"psy_{b}_{h}_{ci}")
                        # denominator partials split across DVE (even jt,
                        # f32r) and GPSIMD (odd jt, f32 - POOL is idle)
                        acc = acc_pool.tile([128, 512], F32R, tag="acc",
                                            name=f"acc_{b}_{h}_{ci}")
                        accB = acc_pool.tile([128, 512], F32R, tag="accB",
                                             name=f"accB_{b}_{h}_{ci}")
                        for jt in range(nj):
                            sub = jt - 4 * ci
                            s0 = max(0, 128 * sub)
                            pt = pt_pool.tile([128, 512], F32R, tag="pt",
                                              name=f"pt_{b}_{h}_{ci}_{jt}")
                            ps_s = psA.tile([128, 512], F32, tag="s",
                                            name=f"pss_{b}_{h}_{ci}_{jt}")
                            nc.tensor.matmul(
                                ps_s[:, s0:512],
                                kT[:, 128 * jt : 128 * (jt + 1)],
                                qT[:, i0 + s0 : i0 + 512],
                                start=True, stop=True,
                            )
                            nc.scalar.activation(
                                pt[:, s0:512], ps_s[:, s0:512], AF.Exp)
                            if sub >= 0:
                                nc.vector.tensor_mul(
                                    pt[:, s0 : s0 + 128],
                                    _f(pt[:, s0 : s0 + 128]),
                                    tri_s[:],
                                )
                            # jt==0 always has s0==0, so start=True initializes
                            # the full 512 columns; later partial-width matmuls
                            # accumulate into their valid subrange only.
                            nc.tensor.matmul(
                                ps_y[:, s0:512],
                                v_s[:, 256 * jt + 128 * h : 256 * jt + 128 * (h + 1)],
                                pt[:, s0:512],
                                start=(jt == 0), stop=(jt == nj - 1),
                            )
                            if jt == 0:
                                nc.vector.tensor_copy(acc[:], _f(pt[:]))
                            elif jt == 1:
                                nc.gpsimd.tensor_copy(
                                    accB[:, s0:512], _f(pt[:, s0:512]))
                            elif jt % 2 == 0:
                                nc.vector.tensor_add(
                                    acc[:, s0:512],
                                    _f(acc[:, s0:512]),
                                    _f(pt[:, s0:512]),
                                )
                            else:
                                nc.gpsimd.tensor_add(
                                    accB[:, s0:512],
                                    _f(accB[:, s0:512]),
                                    _f(pt[:, s0:512]),
                                )
                        # normalize: yt = ps_y * (1/denom) broadcast
                        ps_dd = psA.tile([128, 512], F32, tag="s",
                                         name=f"psdd_{b}_{h}_{ci}")
                        nc.tensor.matmul(
                            ps_dd[0:1, :], ones_s[:, 0:1], acc[:],
                            start=True, stop=False,
                        )
                        sB = 128 if ci == 0 else 0  # accB cols written
                        nc.tensor.matmul(
                            ps_dd[0:1, sB:512], ones_s[:, 0:1], accB[:, sB:512],
                            start=False, stop=True,
                        )
                        rec = small_pool.tile([1, 512], F32R, tag="rec",
                                              name=f"rec_{b}_{h}_{ci}")
                        with nc.allow_low_precision("f32r matmul input"):
                            nc.vector.reciprocal(rec[:], ps_dd[0:1, :])
                        ps_b = psA.tile([128, 512], F32, tag="s",
                                        name=f"psb_{b}_{h}_{ci}")
                        nc.tensor.matmul(
                            ps_b[:], ones_s[0:1, :], rec[:],
                            start=True, stop=True,
                        )
                        ystage = pt_pool.tile([128, 512], F32, tag="pt",
                                              name=f"yst_{b}_{h}_{ci}")
                        nc.scalar.copy(ystage[:], ps_y[:])
                        nc.vector.tensor_mul(
                            yt[h][:, i0 : i0 + 512], ystage[:], ps_b[:]
                        )

                # phase 3 (interleaved): out projection for a tt group
                def emit_outproj(ci, b=b, t0=t0, yt=yt):
                    for tt in range(4 * ci, 4 * ci + 4):
                        for mc in range(4):
                            m0 = 512 * mc
                            ps_o = psA.tile([128, 512], F32, tag="s",
                                            name=f"pso_{b}_{tt}_{mc}")
                            for h in range(HPC):
                                nc.tensor.matmul(
                                    ps_o[:],
                                    yt[h][:, 128 * tt : 128 * (tt + 1)],
                                    wo_s[:, 2048 * h + m0 : 2048 * h + m0 + 512],
                                    start=(h == 0), stop=(h == HPC - 1),
                                )
                            stage = pt_pool.tile([128, 512], F32, tag="pt",
                                                 name=f"st_{b}_{tt}_{mc}")
                            if mc % 2 == 0:
                                nc.scalar.copy(stage[:], ps_o[:])
                            else:
                                nc.vector.tensor_copy(stage[:], ps_o[:])
                            eng = nc.gpsimd if mc % 2 == 0 else nc.scalar
                            eng.dma_start(
                                out_d[t0 + 128 * tt : t0 + 128 * (tt + 1),
                                      m0 : m0 + 512],
                                stage[:],
                            )

                # interleave: two phase-1 chunks + rope group, then both
                # heads' SDPA on the freshly completed i-group, then the
                # out-projection rows that group unlocked
                if b == 0:
                    load_wqk_j(0)
                    nc.sync.dma_start(bqk_s[:], bqk_d[:])
                    xq = {0: emit_x(0)}
                    load_wqk_j(1)
                    load_wqk_j(2)
                    load_wqk_j(3)
                else:
                    xq = {0: emit_x(0)}
                for ci in range(4):
                    emit_chunk(ci, xq.pop(ci))
                    if ci < 3:
                        xq[ci + 1] = emit_x(ci + 1)
                    if b == 0 and ci == 0:
                        load_consts_c()
                    for h in range(HPC):
                        emit_sdpa(h, ci)
                    if b == 0 and ci == 0:
                        load_wo()
                    emit_outproj(ci)
    nc.compile()
    return nc


def _host_prep(x, w_qkv, b_qkv, w_out, b_out):
    """Build per-core input maps."""
    x2 = round_f32r(np.ascontiguousarray(x.reshape(T, H).T))  # [H, T]

    # rope tables (positions repeat per batch -> [128, S])
    inv = 10000.0 ** (-(np.arange(64, dtype=np.float64)) / 64.0)
    ang = np.arange(S, dtype=np.float64)[:, None] * inv[None, :]  # [S, 64]
    cos = np.cos(ang).T.astype(np.float32)  # [64, S]
    sin = np.sin(ang).T.astype(np.float32)
    tab1 = np.concatenate([cos, cos], axis=0)            # [128, S]
    tab2 = np.concatenate([-sin, sin], axis=0)           # [128, S]

    tri = np.triu(np.ones((128, 128), np.float32))       # [j, i] valid i>=j

    perm = np.concatenate([np.arange(0, 128, 2), np.arange(1, 128, 2)])
    scale = 1.0 / np.sqrt(HD)

    wq = w_qkv[0 * H : 1 * H].reshape(NH, HD, H)
    wk = w_qkv[1 * H : 2 * H].reshape(NH, HD, H)
    wv = w_qkv[2 * H : 3 * H].reshape(NH, HD, H)
    bq = b_qkv[0 * H : 1 * H].reshape(NH, HD)
    bk = b_qkv[1 * H : 2 * H].reshape(NH, HD)
    bv = b_qkv[2 * H : 3 * H].reshape(NH, HD)

    in_maps = []
    for c in range(NCORES):
        hs = [HPC * c + i for i in range(HPC)]
        cols = []
        bcols = []
        for h in hs:  # q heads (scaled + permuted)
            cols.append(wq[h][perm] * scale)
            bcols.append(bq[h][perm] * scale)
        for h in hs:  # k heads (permuted)
            cols.append(wk[h][perm])
            bcols.append(bk[h][perm])
        wqkT = round_f32r(
            np.ascontiguousarray(np.concatenate(cols, axis=0).T.astype(np.float32))
        )
        bqk = np.stack(bcols, axis=1).astype(np.float32)  # [128, 4]

        wvT = round_f32r(np.ascontiguousarray(
            np.concatenate([wv[h] for h in hs], axis=0).T.astype(np.float32)
        ))  # [H, 256]
        bvb = np.tile(
            np.concatenate([bv[h] for h in hs])[None, :], (128, 1)
        ).astype(np.float32)  # [128, 256]

        woutT = round_f32r(np.ascontiguousarray(
            w_out[:, HD * hs[0] : HD * (hs[-1] + 1)].T.astype(np.float32)
        ))  # [256, H]

        in_maps.append({
            "xT": x2,
            "wqkT": wqkT,
            "wvT": wvT,
            "woutT": woutT,
            "bqk": bqk,
            "bv": bvb,
            "tab1": tab1,
            "tab2": tab2,
            "tri": tri,
            "ones": np.ones((128, 128), np.float32),
        })
    return in_maps


_NC_CACHE = None


def get_module():
    global _NC_CACHE
    if _NC_CACHE is None:
        _NC_CACHE = build_module()
    return _NC_CACHE


def kernel(x, w_qkv, b_qkv, w_out, b_out):
    x = np.asarray(x, dtype=np.float32)
    w_qkv = np.asarray(w_qkv, dtype=np.float32)
    b_qkv = np.asarray(b_qkv, dtype=np.float32)
    w_out = np.asarray(w_out, dtype=np.float32)
    b_out = np.asarray(b_out, dtype=np.float32)

    nc = get_module()
    in_maps = _host_prep(x, w_qkv, b_qkv, w_out, b_out)
    res = run_bass_kernel_spmd(nc, in_maps, core_ids=list(range(NCORES)))
    acc = np.zeros((T, H), np.float64)
    for c in range(NCORES):
        acc += res.results[c]["outp"].astype(np.float64)
    out = (acc + b_out.astype(np.float64)[None, :]).astype(np.float32)
    return out.reshape(B, S, H)


# revision 64
# speedup vs baseline: 1.3363x; 1.0314x over previous
"""Causal self-attention (B=2, S=2048, H=2048, 16 heads, hd=128) on 8 trn2 cores.

Sharding: tensor-parallel over heads (2 heads/core). Each core computes its
heads' QKV projection + RoPE + causal SDPA + a partial out-projection
(row-parallel w_out); the all-reduce over cores is done host-side as the
unshard step (sum of partials + b_out).

Device kernel (identical program on all cores, per-core weight data):
  phase 1: qkv = x @ w^T per head, q/k emitted head-major [d, t] via
           lhsT=w^T tiles, v emitted natural [t, d] via lhsT=x^T tiles.
           float32r matmuls (full PE rate, fp32 storage, 11-bit mantissa).
  rope:    even/odd dims pre-permuted into [evens;odds] rows host-side, so
           RoPE = pre*tab1 + swap(pre)*tab2 with a partition half-swap DMA.
  sdpa:    scores^T blocks [j,128 x i,512] = k^T.T @ q^T, exp on ScalarE,
           causal via N-restricted matmuls + triangular mask multiply,
           y^T accumulated in PSUM (lhsT=v_j), denominator via ones-matmul,
           normalize with reciprocal + K=1 broadcast matmul.
  phase 3: partial out [t,m] += y^T.T @ w_out^T slices, DMA to DRAM.

All matmul inputs are float32r: DRAM-sourced arrays are pre-rounded on the
host (RNE to 11-bit mantissa, low 12 bits zeroed); device-produced tiles are
declared f32r so ACT/DVE round on write; compute reads go through a .bitcast
back to f32.
"""

import sys

for _p in ("/opt/trn_rl_repo",):
    if _p not in sys.path:
        sys.path.append(_p)

import numpy as np

import concourse.bass as bass
import concourse.tile as tile
from concourse import bacc, mybir
from concourse.bass_utils import run_bass_kernel_spmd

B = 2
S = 2048
H = 2048
NH = 16
HD = 128
NCORES = 8
HPC = NH // NCORES          # heads per core = 2
T = B * S                   # 4096 flattened tokens
CHUNK = 512                 # phase-1 t-chunk (= one SDPA i-group)
NCHUNK = S // CHUNK         # 4 per batch
F32 = mybir.dt.float32
F32R = mybir.dt.float32r
AF = mybir.ActivationFunctionType


def _f(ap):
    """View an f32r AP as plain f32 for compute-engine reads."""
    return ap.bitcast(F32)


def round_f32r(a: np.ndarray) -> np.ndarray:
    """Round fp32 to f32r (RNE to 11 explicit mantissa bits, low 12 bits 0)."""
    u = a.astype(np.float32).view(np.uint32)
    r = u + (0x7FF + ((u >> 12) & 1))
    return (r & np.uint32(0xFFFFF000)).view(np.float32)


def build_module():
    nc = bacc.Bacc("TRN2", target_bir_lowering=False, debug=False)

    xT_d = nc.dram_tensor("xT", [H, T], F32R, kind="ExternalInput").ap()
    wqk_d = nc.dram_tensor("wqkT", [H, 4 * HD], F32R, kind="ExternalInput").ap()
    wv_d = nc.dram_tensor("wvT", [H, 2 * HD], F32R, kind="ExternalInput").ap()
    wout_d = nc.dram_tensor("woutT", [2 * HD, H], F32R, kind="ExternalInput").ap()
    bqk_d = nc.dram_tensor("bqk", [128, 4], F32, kind="ExternalInput").ap()
    bv_d = nc.dram_tensor("bv", [128, 2 * HD], F32, kind="ExternalInput").ap()
    tab1_d = nc.dram_tensor("tab1", [128, S], F32, kind="ExternalInput").ap()
    tab2_d = nc.dram_tensor("tab2", [128, S], F32, kind="ExternalInput").ap()
    tri_d = nc.dram_tensor("tri", [128, 128], F32, kind="ExternalInput").ap()
    ones_d = nc.dram_tensor("ones", [128, 128], F32R, kind="ExternalInput").ap()
    out_d = nc.dram_tensor("outp", [T, H], F32, kind="ExternalOutput").ap()

    with tile.TileContext(nc) as tc:
        with (
            tc.tile_pool(name="consts", bufs=1) as consts,
            tc.tile_pool(name="qkpre", bufs=1) as qkpre_pool,
            tc.tile_pool(name="vpool", bufs=1) as v_pool,
            tc.tile_pool(name="ytpool", bufs=1) as yt_pool,
            tc.tile_pool(name="xpool", bufs=8) as x_pool,
            tc.tile_pool(name="rope", bufs=4) as rope_pool,
            tc.tile_pool(name="ptpool", bufs=7) as pt_pool,
            tc.tile_pool(name="small", bufs=1) as small_pool,
            tc.tile_pool(name="accpool", bufs=1) as acc_pool,
            tc.tile_pool(name="ps256", bufs=4, space="PSUM") as ps256,
            tc.tile_pool(name="psA", bufs=3, space="PSUM") as psA,
            tc.tile_pool(name="psY", bufs=1, space="PSUM") as psY,
        ):
            # ---- resident constants ----
            wqk_s = consts.tile([128, 16 * 512], F32R, name="wqk_s")

            def load_wqk_j(j):
                nc.sync.dma_start(
                    wqk_s[:].rearrange("p (g o) -> p g o", g=16)[
                        :, :, 128 * j : 128 * (j + 1)],
                    wqk_d.rearrange("(g p) o -> p g o", p=128)[
                        :, :, 128 * j : 128 * (j + 1)],
                )
            wv_s = consts.tile([128, 16 * 256], F32R, name="wv_s")
            bqk_s = consts.tile([128, 4], F32, name="bqk_s")
            bv_s = consts.tile([128, 2 * HD], F32, name="bv_s")
            # loaded later (first needed once chunk 0's matmuls are running)
            # so the startup DMA burst only covers wqk + the first x chunks
            tab1_s = consts.tile([128, S], F32, name="tab1_s")
            tab2_s = consts.tile([128, S], F32, name="tab2_s")
            tri_s = consts.tile([128, 128], F32, name="tri_s")
            ones_s = consts.tile([128, 128], F32R, name="ones_s")
            wo_s = consts.tile([128, 2 * H], F32R, name="wo_s")

            def load_consts_a():
                nc.sync.dma_start(
                    wv_s[:].rearrange("p (g o) -> p g o", g=16),
                    wv_d.rearrange("(g p) o -> p g o", p=128),
                )
                nc.sync.dma_start(bv_s[:], bv_d[:])

            def load_consts_b():
                nc.sync.dma_start(tab1_s[:], tab1_d[:])
                nc.sync.dma_start(tab2_s[:], tab2_d[:])

            def load_consts_c():
                nc.sync.dma_start(tri_s[:], tri_d[:])
                nc.sync.dma_start(ones_s[:], ones_d[:])

            def load_wo():
                nc.sync.dma_start(
                    wo_s[:].rearrange("p (g m) -> p g m", g=2),
                    wout_d.rearrange("(g p) m -> p g m", p=128),
                )

            for b in range(B):
                t0 = b * S

                # j order: q_h0, q_h1, k_h0, k_h1
                pre = [
                    qkpre_pool.tile([128, S], F32R, tag=f"pre{j}", name=f"pre{j}_{b}")
                    for j in range(4)
                ]
                v_s = v_pool.tile([128, 16 * 256], F32R, tag="v", name=f"v_{b}")
                yt = [
                    yt_pool.tile([128, S], F32R, tag=f"yt{h}", name=f"yt{h}_{b}")
                    for h in range(HPC)
                ]

                # ---------------- phase 1: QKV projection ----------------
                def emit_x(c, b=b, t0=t0):
                    tc_off = t0 + CHUNK * c
                    xh = []
                    xT_3d = xT_d.rearrange("(g p) t -> p g t", p=128)
                    for e in range(8):
                        xt = x_pool.tile([128, 2 * CHUNK], F32R, tag="xc",
                                         name=f"xc_{b}_{c}_{e}")
                        nc.sync.dma_start(
                            xt[:].rearrange("p (g c) -> p g c", g=2),
                            xT_3d[:, 2 * e : 2 * (e + 1),
                                  tc_off : tc_off + CHUNK],
                        )
                        xh.append(xt)
                    return xh

                def emit_chunk(c, xh, b=b, t0=t0, pre=pre, v_s=v_s):
                    for j in range(4):
                        ps = ps256.tile([128, CHUNK], F32, tag="p1",
                                        name=f"psqk_{b}_{c}_{j}")
                        for ht in range(16):
                            nc.tensor.matmul(
                                ps[:],
                                wqk_s[:, 512 * ht + 128 * j : 512 * ht + 128 * (j + 1)],
                                xh[ht // 2][:, CHUNK * (ht % 2) : CHUNK * (ht % 2 + 1)],
                                start=(ht == 0),
                                stop=(ht == 15),
                            )
                        nc.scalar.activation(
                            pre[j][:, CHUNK * c : CHUNK * (c + 1)],
                            ps[:],
                            AF.Identity,
                            bias=bqk_s[:, j : j + 1],
                        )

                    for tt in range(4):
                        gtt = 4 * c + tt  # t-tile within batch
                        psv = ps256.tile([128, 256], F32, tag="p1",
                                         name=f"psv_{b}_{c}_{tt}")
                        for ht in range(16):
                            nc.tensor.matmul(
                                psv[:],
                                xh[ht // 2][:, CHUNK * (ht % 2) + 128 * tt :
                                            CHUNK * (ht % 2) + 128 * (tt + 1)],
                                wv_s[:, 256 * ht : 256 * (ht + 1)],
                                start=(ht == 0),
                                stop=(ht == 15),
                            )
                        nc.vector.tensor_add(
                            v_s[:, 256 * gtt : 256 * (gtt + 1)], psv[:], bv_s[:]
                        )

                    if b == 0 and c == 0:
                        load_consts_a()
                    if b == 0 and c == 0:
                        load_consts_b()

                    # rope for this 512-column chunk
                    if True:
                        cs = slice(CHUNK * c, CHUNK * (c + 1))
                        for j in range(4):
                            swap = rope_pool.tile([128, 512], F32R, tag="swap",
                                                  name=f"swap_{b}_{c}_{j}")
                            nc.scalar.dma_start(swap[0:64, :], pre[j][64:128, cs])
                            nc.scalar.dma_start(swap[64:128, :], pre[j][0:64, cs])
                            nc.vector.tensor_mul(
                                swap[:], _f(swap[:]), tab2_s[:, cs])
                            nc.vector.tensor_mul(
                                pre[j][:, cs], _f(pre[j][:, cs]), tab1_s[:, cs])
                            nc.vector.tensor_add(
                                pre[j][:, cs], _f(pre[j][:, cs]), _f(swap[:]))

                # ---------------- phase 2: SDPA (interleaved per i-group) ---
                def emit_sdpa(h, ci, b=b, rot=pre, v_s=v_s, yt=yt):
                    qT = rot[h]
                    kT = rot[2 + h]
                    if True:
                        i0 = 512 * ci
                        nj = 4 * ci + 4
                        ps_y = psY.tile([128, 512], F32, tag="y",
                                        name=f"psy_{b}_{h}_{ci}")
                        # denominator partials split across DVE (even jt,
                        # f32r) and GPSIMD (odd jt, f32 - POOL is idle)
                        acc = acc_pool.tile([128, 512], F32R, tag="acc",
                                            name=f"acc_{b}_{h}_{ci}")
                        accB = acc_pool.tile([128, 512], F32R, tag="accB",
                                             name=f"accB_{b}_{h}_{ci}")
                        for jt in range(nj):
                            sub = jt - 4 * ci
                            s0 = max(0, 128 * sub)
                            pt = pt_pool.tile([128, 512], F32R, tag="pt",
                                              name=f"pt_{b}_{h}_{ci}_{jt}")
                            ps_s = psA.tile([128, 512], F32, tag="s",
                                            name=f"pss_{b}_{h}_{ci}_{jt}")
                            nc.tensor.matmul(
                                ps_s[:, s0:512],
                                kT[:, 128 * jt : 128 * (jt + 1)],
                                qT[:, i0 + s0 : i0 + 512],
                                start=True, stop=True,
                            )
                            nc.scalar.activation(
                                pt[:, s0:512], ps_s[:, s0:512], AF.Exp)
                            if sub >= 0:
                                nc.vector.tensor_mul(
                                    pt[:, s0 : s0 + 128],
                                    _f(pt[:, s0 : s0 + 128]),
                                    tri_s[:],
                                )
                            # jt==0 always has s0==0, so start=True initializes
                            # the full 512 columns; later partial-width matmuls
                            # accumulate into their valid subrange only.
                            nc.tensor.matmul(
                                ps_y[:, s0:512],
                                v_s[:, 256 * jt + 128 * h : 256 * jt + 128 * (h + 1)],
                                pt[:, s0:512],
                                start=(jt == 0), stop=(jt == nj - 1),
                            )
                            if jt == 0:
                                nc.vector.tensor_copy(acc[:], _f(pt[:]))
                            elif jt == 1:
                                nc.gpsimd.tensor_copy(
                                    accB[:, s0:512], _f(pt[:, s0:512]))
                            elif jt % 2 == 0:
                                nc.vector.tensor_add(
                                    acc[:, s0:512],
                                    _f(acc[:, s0:512]),
                                    _f(pt[:, s0:512]),
                                )
                            else:
                                nc.gpsimd.tensor_add(
                                    accB[:, s0:512],
                                    _f(accB[:, s0:512]),
                                    _f(pt[:, s0:512]),
                                )
                        # normalize: yt = ps_y * (1/denom) broadcast
                        ps_dd = psA.tile([128, 512], F32, tag="s",
                                         name=f"psdd_{b}_{h}_{ci}")
                        nc.tensor.matmul(
                            ps_dd[0:1, :], ones_s[:, 0:1], acc[:],
                            start=True, stop=False,
                        )
                        sB = 128 if ci == 0 else 0  # accB cols written
                        nc.tensor.matmul(
                            ps_dd[0:1, sB:512], ones_s[:, 0:1], accB[:, sB:512],
                            start=False, stop=True,
                        )
                        rec = small_pool.tile([1, 512], F32R, tag="rec",
                                              name=f"rec_{b}_{h}_{ci}")
                        with nc.allow_low_precision("f32r matmul input"):
                            nc.vector.reciprocal(rec[:], ps_dd[0:1, :])
                        ps_b = psA.tile([128, 512], F32, tag="s",
                                        name=f"psb_{b}_{h}_{ci}")
                        nc.tensor.matmul(
                            ps_b[:], ones_s[0:1, :], rec[:],
                            start=True, stop=True,
                        )
                        ystage = pt_pool.tile([128, 512], F32, tag="pt",
                                              name=f"yst_{b}_{h}_{ci}")
                        nc.scalar.copy(ystage[:], ps_y[:])
                        nc.vector.tensor_mul(
                            yt[h][:, i0 : i0 + 512], ystage[:], ps_b[:]
                        )

                # phase 3 (interleaved): out projection for a tt group
                def emit_outproj(ci, b=b, t0=t0, yt=yt):
                    for tt in range(4 * ci, 4 * ci + 4):
                        for mc in range(4):
                            m0 = 512 * mc
                            ps_o = psA.tile([128, 512], F32, tag="s",
                                            name=f"pso_{b}_{tt}_{mc}")
                            for h in range(HPC):
                                nc.tensor.matmul(
                                    ps_o[:],
                                    yt[h][:, 128 * tt : 128 * (tt + 1)],
                                    wo_s[:, 2048 * h + m0 : 2048 * h + m0 + 512],
                                    start=(h == 0), stop=(h == HPC - 1),
                                )
                            stage = pt_pool.tile([128, 512], F32, tag="pt",
                                                 name=f"st_{b}_{tt}_{mc}")
                            if mc % 2 == 0:
                                nc.scalar.copy(stage[:], ps_o[:])
                            else:
                                nc.vector.tensor_copy(stage[:], ps_o[:])
                            eng = nc.gpsimd if mc % 2 == 0 else nc.scalar
                            eng.dma_start(
                                out_d[t0 + 128 * tt : t0 + 128 * (tt + 1),
                                      m0 : m0 + 512],
                                stage[:],
                            )

                # interleave: two phase-1 chunks + rope group, then both
                # heads' SDPA on the freshly completed i-group, then the
                # out-projection rows that group unlocked
                if b == 0:
                    load_wqk_j(0)
                    nc.sync.dma_start(bqk_s[:], bqk_d[:])
                    xq = {0: emit_x(0)}
                    load_wqk_j(1)
                    load_wqk_j(2)
                    load_wqk_j(3)
                else:
                    xq = {0: emit_x(0)}
                for ci in range(4):
                    emit_chunk(ci, xq.pop(ci))
                    if ci < 3:
                        xq[ci + 1] = emit_x(ci + 1)
                    if b == 0 and ci == 0:
                        load_consts_c()
                    for h in range(HPC):
                        emit_sdpa(h, ci)
                    if b == 0 and ci == 0:
                        load_wo()
                    emit_outproj(ci)
    nc.compile()
    return nc


def _host_prep(x, w_qkv, b_qkv, w_out, b_out):
    """Build per-core input maps."""
    x2 = round_f32r(np.ascontiguousarray(x.reshape(T, H).T))  # [H, T]

    # rope tables (positions repeat per batch -> [128, S])
    inv = 10000.0 ** (-(np.arange(64, dtype=np.float64)) / 64.0)
    ang = np.arange(S, dtype=np.float64)[:, None] * inv[None, :]  # [S, 64]
    cos = np.cos(ang).T.astype(np.float32)  # [64, S]
    sin = np.sin(ang).T.astype(np.float32)
    tab1 = np.concatenate([cos, cos], axis=0)            # [128, S]
    tab2 = np.concatenate([-sin, sin], axis=0)           # [128, S]

    tri = np.triu(np.ones((128, 128), np.float32))       # [j, i] valid i>=j

    perm = np.concatenate([np.arange(0, 128, 2), np.arange(1, 128, 2)])
    scale = 1.0 / np.sqrt(HD)

    wq = w_qkv[0 * H : 1 * H].reshape(NH, HD, H)
    wk = w_qkv[1 * H : 2 * H].reshape(NH, HD, H)
    wv = w_qkv[2 * H : 3 * H].reshape(NH, HD, H)
    bq = b_qkv[0 * H : 1 * H].reshape(NH, HD)
    bk = b_qkv[1 * H : 2 * H].reshape(NH, HD)
    bv = b_qkv[2 * H : 3 * H].reshape(NH, HD)

    in_maps = []
    for c in range(NCORES):
        hs = [HPC * c + i for i in range(HPC)]
        cols = []
        bcols = []
        for h in hs:  # q heads (scaled + permuted)
            cols.append(wq[h][perm] * scale)
            bcols.append(bq[h][perm] * scale)
        for h in hs:  # k heads (permuted)
            cols.append(wk[h][perm])
            bcols.append(bk[h][perm])
        wqkT = round_f32r(
            np.ascontiguousarray(np.concatenate(cols, axis=0).T.astype(np.float32))
        )
        bqk = np.stack(bcols, axis=1).astype(np.float32)  # [128, 4]

        wvT = round_f32r(np.ascontiguousarray(
            np.concatenate([wv[h] for h in hs], axis=0).T.astype(np.float32)
        ))  # [H, 256]
        bvb = np.tile(
            np.concatenate([bv[h] for h in hs])[None, :], (128, 1)
        ).astype(np.float32)  # [128, 256]

        woutT = round_f32r(np.ascontiguousarray(
            w_out[:, HD * hs[0] : HD * (hs[-1] + 1)].T.astype(np.float32)
        ))  # [256, H]

        in_maps.append({
            "xT": x2,
            "wqkT": wqkT,
            "wvT": wvT,
            "woutT": woutT,
            "bqk": bqk,
            "bv": bvb,
            "tab1": tab1,
            "tab2": tab2,
            "tri": tri,
            "ones": np.ones((128, 128), np.float32),
        })
    return in_maps


_NC_CACHE = None


def get_module():
    global _NC_CACHE
    if _NC_CACHE is None:
        _NC_CACHE = build_module()
    return _NC_CACHE


def kernel(x, w_qkv, b_qkv, w_out, b_out):
    x = np.asarray(x, dtype=np.float32)
    w_qkv = np.asarray(w_qkv, dtype=np.float32)
    b_qkv = np.asarray(b_qkv, dtype=np.float32)
    w_out = np.asarray(w_out, dtype=np.float32)
    b_out = np.asarray(b_out, dtype=np.float32)

    nc = get_module()
    in_maps = _host_prep(x, w_qkv, b_qkv, w_out, b_out)
    res = run_bass_kernel_spmd(nc, in_maps, core_ids=list(range(NCORES)))
    acc = np.zeros((T, H), np.float64)
    for c in range(NCORES):
        acc += res.results[c]["outp"].astype(np.float64)
    out = (acc + b_out.astype(np.float64)[None, :]).astype(np.float32)
    return out.reshape(B, S, H)
